# revision 16
# baseline (speedup 1.0000x reference)
"""EnhancedMultiHeadAttention on 8 TRN2 NeuronCores.

Sharding: core c handles batch b=c//2 and query-row half h=c%2.
Each core computes the full attention for its 1024 query rows against its
batch's full 2048 keys/values. Outputs are disjoint slices of the full
[4, 2048, 1024] result, assembled on the host.

Host-side prep: LN gain/beta are folded into the q/k/v/gate projection
weights and biases in numpy (W' = diag(g) @ W, b' = beta @ W + b); weights
are shipped to the device in bf16 already laid out as [128, KT, D]
(partition = input-dim within contraction tile). Activations ship as bf16.

Device kernel structure (bf16 matmuls, f32 softmax/LN):
  - LayerNorm in token-major layout (DVE stats, rstd = exp(-0.5*log(var+eps))
    so ACT only ever needs the exp/log table set); normalized bf16 tiles are
    transposed via DMA-XBAR (128x128 tiles) straight into contraction-tile
    layout -- no PE transposes, no PSUM staging.
  - K proj is chunk-major, Q proj m-chunk-major, so pair 0's scores unlock
    as early as possible.  V proj is interleaved into pair 0's attention
    window (its PSUM tiles rotate through the score pool before any pO
    tile exists).
  - Scores are computed transposed [Sk, Sq] with the two heads of an
    m-chunk row-packed as concurrent K=64 matmuls in disjoint row-group
    halves of the PE array (the layout already places head 2m in
    partitions 0-63 and head 2m+1 in 64-127).
  - exp (no max subtraction -- scores ~N(0,1)) writes per-sk bf16 E^T
    tiles into a small ring; A@V uses v (with a ones column appended per
    head) as the stationary operand so the softmax denominator falls out
    as psum row 64; denominators inverted with reciprocal_approx_fast,
    broadcast via gpsimd, applied on DVE writing attn_out^T directly in
    the out-proj layout.
  - out-proj + gate folded into the final LN exactly as before, with the
    big per-element passes on DVE instead of ACT.
"""

import os
import numpy as np

D = 1024
H = 16
HD = 64
S = 2048
B = 4
SQ = 1024  # query rows per core
SK = 2048  # kv rows per core
KT = D // 128  # contraction tiles
MT = D // 128  # output chunks
N_CORES = 8
EPS = 1e-5
LAG = 1  # A@V lag (in sk tiles) behind exp during pair 0

_CACHE = {}


def _build(triv_b=False, triv_v=False, triv_o=False, triv_lno=False):
    """triv_b: folded q/k/gate biases all zero; triv_v: folded v bias zero;
    triv_o: bo zero; triv_lno: final LN gain==1 and beta==0."""
    from contextlib import ExitStack

    import concourse.bacc as bacc
    import concourse.bass as bass
    import concourse.mybir as mybir
    import concourse.tile as tile

    f32 = mybir.dt.float32
    bf16 = mybir.dt.bfloat16
    AF = mybir.ActivationFunctionType
    OP = mybir.AluOpType

    nc = bacc.Bacc("TRN2", target_bir_lowering=False, debug=False,
                   num_devices=N_CORES)

    xq = nc.dram_tensor("xq", [SQ, D], bf16, kind="ExternalInput").ap()
    xk = nc.dram_tensor("xk", [SK, D], bf16, kind="ExternalInput").ap()
    xv = nc.dram_tensor("xv", [SK, D], bf16, kind="ExternalInput").ap()
    # weights pre-folded + pre-laid-out on host: [128, KT*D] bf16
    wq_d = nc.dram_tensor("wq", [128, KT * D], bf16, kind="ExternalInput").ap()
    wk_d = nc.dram_tensor("wk", [128, KT * D], bf16, kind="ExternalInput").ap()
    wv_d = nc.dram_tensor("wv", [128, KT * D], bf16, kind="ExternalInput").ap()
    wo_d = nc.dram_tensor("wo", [128, KT * D], bf16, kind="ExternalInput").ap()
    wg_d = nc.dram_tensor("wg", [128, KT], bf16, kind="ExternalInput").ap()
    # biases (already folded on host)
    bq_d = nc.dram_tensor("bqc", [128, MT], f32, kind="ExternalInput").ap()
    bk_d = nc.dram_tensor("bkc", [128, MT], f32, kind="ExternalInput").ap()
    bv_d = nc.dram_tensor("bvr", [1, D], bf16, kind="ExternalInput").ap()
    bo_d = nc.dram_tensor("bor", [1, D], bf16, kind="ExternalInput").ap()
    bg_d = nc.dram_tensor("bgs", [1, 1], bf16, kind="ExternalInput").ap()
    lnog_d = nc.dram_tensor("lnog", [1, D], f32, kind="ExternalInput").ap()
    lnob_d = nc.dram_tensor("lnob", [1, D], f32, kind="ExternalInput").ap()
    out_d = nc.dram_tensor("out", [SQ, D], f32, kind="ExternalOutput").ap()
    dbg = os.environ.get("KDBG")
    if dbg:
        dq_d = nc.dram_tensor("dbg_qT", [128, MT * SQ], f32,
                              kind="ExternalOutput").ap()
        dk_d = nc.dram_tensor("dbg_kT", [128, MT * SK], f32,
                              kind="ExternalOutput").ap()
        dv_d = nc.dram_tensor("dbg_v", [128, (SK // 128) * H * (HD + 1)], f32,
                              kind="ExternalOutput").ap()
        da_d = nc.dram_tensor("dbg_aT", [128, KT * SQ], f32,
                              kind="ExternalOutput").ap()

    def bcast_rows(ap2d, p):
        return bass.AP(tensor=ap2d.tensor, offset=ap2d.offset,
                       ap=[[0, p]] + list(ap2d.ap[1:]))

    with tile.TileContext(nc) as tc:
        with ExitStack() as ctx:
            const = ctx.enter_context(tc.tile_pool(name="const", bufs=1))
            main = ctx.enter_context(tc.tile_pool(name="main", bufs=1))
            wpl = ctx.enter_context(tc.tile_pool(name="wpl", bufs=2))
            xnt = ctx.enter_context(tc.tile_pool(name="xnt", bufs=2))
            lnw = ctx.enter_context(tc.tile_pool(name="lnw", bufs=3))
            etp = ctx.enter_context(tc.tile_pool(name="etp", bufs=4))
            dvp = ctx.enter_context(tc.tile_pool(name="dvp", bufs=2))
            psA = ctx.enter_context(
                tc.tile_pool(name="psA", bufs=2, space="PSUM"))
            psB = ctx.enter_context(
                tc.tile_pool(name="psB", bufs=2, space="PSUM"))

            eps_t = const.tile([128, 1], f32)
            nc.vector.memset(eps_t, EPS)
            ones_row = const.tile([1, 128], bf16)
            nc.vector.memset(ones_row, 1.0)

            # persistent per-core intermediates
            kT_s = main.tile([128, MT, SK], bf16)
            qT_s = main.tile([128, MT, SQ], bf16)
            v_aug = main.tile([128, SK // 128, H, HD + 1], bf16)
            attn_oT = main.tile([128, KT, SQ], bf16)
            gate_s = main.tile([128, SQ // 128], f32)
            eg_s = main.tile([128, SQ // 128], f32)
            nc.vector.memset(v_aug[:, :, :, HD:HD + 1], 1.0)

            if not triv_b:
                bqc = const.tile([128, MT], f32)
                nc.sync.dma_start(out=bqc, in_=bq_d)
                bkc = const.tile([128, MT], f32)
                nc.sync.dma_start(out=bkc, in_=bk_d)
                bg_s = const.tile([1, 1], bf16)
                nc.sync.dma_start(out=bg_s, in_=bg_d)
            if not triv_v:
                bvr = const.tile([1, D], bf16)
                nc.sync.dma_start(out=bvr, in_=bv_d)
                bvb = const.tile([128, D], bf16)
                nc.gpsimd.partition_broadcast(out_ap=bvb, in_ap=bvr)
            if not triv_o:
                bo_s = const.tile([1, D], bf16)
                nc.sync.dma_start(out=bo_s, in_=bo_d)
            wg_s = const.tile([128, KT], bf16)
            nc.sync.dma_start(out=wg_s, in_=wg_d)

            def ln_tile(x_dram, t, chunk):
                """LN (gain/beta folded into W') of token tile t, then
                DMA-XBAR transpose into chunk[:, c, col:col+128]."""
                xt = lnw.tile([128, D], bf16, tag="x", bufs=2)
                nc.sync.dma_start(out=xt,
                                  in_=x_dram[t * 128:(t + 1) * 128, :])
                xt3 = xt.rearrange("p (s f) -> p s f", s=2)
                stats = lnw.tile([128, 2, 6], f32, tag="st")
                nc.vector.bn_stats(out=stats[:, 0, :], in_=xt3[:, 0, :])
                nc.vector.bn_stats(out=stats[:, 1, :], in_=xt3[:, 1, :])
                mv = lnw.tile([128, 2], f32, tag="mv")
                nc.vector.bn_aggr(out=mv, in_=stats)
                # rstd = (var+eps)^-0.5 = exp(-0.5*log(var+eps))
                rstd = lnw.tile([128, 1], f32, tag="rs")
                nc.scalar.activation(out=rstd, in_=mv[:, 1:2],
                                     func=AF.Ln, bias=eps_t)
                nc.scalar.activation(out=rstd, in_=rstd,
                                     func=AF.Exp, scale=-0.5)
                xc = lnw.tile([128, D], bf16, tag="xc", bufs=2)
                nc.vector.tensor_scalar(
                    out=xc, in0=xt, scalar1=mv[:, 0:1], scalar2=rstd,
                    op0=OP.subtract, op1=OP.mult)
                col = (t % 4) * 128
                for c in range(KT):
                    nc.sync.dma_start(
                        out=chunk[:, c, col:col + 128],
                        in_=xc[:, c * 128:(c + 1) * 128], transpose=True)

            def proj_chunk(Ws, bcol, chunk, cc, dst, ms):
                """dst[:, m, cc*512:+512] = (W'^T xn^T + b') for m in ms."""
                for m in ms:
                    ps = psA.tile([128, 512], f32, tag="a")
                    for kt in range(KT):
                        nc.tensor.matmul(
                            out=ps,
                            lhsT=Ws[:, kt, m * 128:(m + 1) * 128],
                            rhs=chunk[:, kt, :],
                            start=(kt == 0), stop=(kt == KT - 1))
                    if bcol is None:
                        nc.vector.tensor_copy(
                            out=dst[:, m, cc * 512:(cc + 1) * 512], in_=ps)
                    else:
                        nc.vector.tensor_scalar_add(
                            out=dst[:, m, cc * 512:(cc + 1) * 512], in0=ps,
                            scalar1=bcol[:, m:m + 1])

            # ---------------- K path (chunk-major) ----------------
            wk_s = wpl.tile([128, KT, D], bf16, tag="w", name="wk")
            nc.sync.dma_start(out=wk_s.rearrange("p a b -> p (a b)"),
                              in_=wk_d)
            for cc in range(4):
                knc = xnt.tile([128, KT, 512], bf16, tag="kn",
                               name=f"knT{cc}")
                for t in range(4 * cc, 4 * cc + 4):
                    ln_tile(xk, t, knc)
                proj_chunk(wk_s, None if triv_b else bkc, knc, cc, kT_s,
                           range(MT))

            # ---------------- Q path (m-major) + gate ----------------
            wq_s = wpl.tile([128, KT, D], bf16, tag="w", name="wq")
            nc.sync.dma_start(out=wq_s.rearrange("p a b -> p (a b)"),
                              in_=wq_d)
            qnc = []
            for cc in range(2):
                c = xnt.tile([128, KT, 512], bf16, tag="qn", name=f"qnT{cc}")
                qnc.append(c)
                for t in range(4 * cc, 4 * cc + 4):
                    ln_tile(xq, t, c)
            for m in range(MT):
                for cc in range(2):
                    proj_chunk(wq_s, None if triv_b else bqc, qnc[cc], cc,
                               qT_s, [m])
            # gate: sigmoid(qn @ Wg') = 1/(1+exp(-(qn@Wg'+bg)))
            for tt in range(SQ // 128):
                gps = psB.tile([128, 1], f32, tag="b")
                for kt in range(KT):
                    nc.tensor.matmul(
                        out=gps,
                        lhsT=qnc[tt // 4][:, kt, (tt % 4) * 128:
                                          (tt % 4 + 1) * 128],
                        rhs=wg_s[:, kt:kt + 1],
                        start=(kt == 0), stop=(kt == KT - 1 and triv_b))
                if not triv_b:
                    nc.tensor.matmul(out=gps, lhsT=ones_row, rhs=bg_s,
                                     start=False, stop=True)
                nc.scalar.activation(out=eg_s[:, tt:tt + 1], in_=gps,
                                     func=AF.Exp, scale=-1.0)
            nc.vector.tensor_scalar_add(out=gate_s, in0=eg_s, scalar1=1.0)
            nc.vector.reciprocal_approx_fast(out=gate_s, in_=gate_s)

            # ---------------- V weights + LN (proj is interleaved) -------
            wv_s = wpl.tile([128, KT, D], bf16, tag="w", name="wv")
            nc.sync.dma_start(out=wv_s.rearrange("p a b -> p (a b)"),
                              in_=wv_d)
            vnc = {}

            def vproj_sk(sk):
                """v_aug[:, sk, :, :HD] = (vn W_v' + b_v) for token tile sk."""
                cc = sk // 4
                if sk % 4 == 0:
                    vnc[cc] = xnt.tile([128, KT, 512], bf16, tag="vn",
                                       name=f"vnT{cc}")
                    for t in range(4 * cc, 4 * cc + 4):
                        ln_tile(xv, t, vnc[cc])
                for n in range(2):
                    ps = psA.tile([128, 512], f32, tag="a")
                    for kt in range(KT):
                        nc.tensor.matmul(
                            out=ps,
                            lhsT=vnc[cc][:, kt,
                                         (sk % 4) * 128:(sk % 4 + 1) * 128],
                            rhs=wv_s[:, kt, n * 512:(n + 1) * 512],
                            start=(kt == 0), stop=(kt == KT - 1))
                    if triv_v:
                        nc.vector.tensor_copy(
                            out=v_aug[:, sk, n * 8:(n + 1) * 8, 0:HD],
                            in_=ps.rearrange("p (h d) -> p h d", h=8))
                    else:
                        nc.vector.scalar_tensor_tensor(
                            out=v_aug[:, sk, n * 8:(n + 1) * 8, 0:HD],
                            in0=ps.rearrange("p (h d) -> p h d", h=8),
                            scalar=1.0, op0=OP.mult, op1=OP.add,
                            in1=bvb[:, n * 512:(n + 1) * 512].rearrange(
                                "p (h d) -> p h d", h=8))

            # load Wo into the slot wk/wv rotation frees later
            wo_s = wpl.tile([128, KT, D], bf16, tag="w", name="wo")
            nc.sync.dma_start(out=wo_s.rearrange("p a b -> p (a b)"),
                              in_=wo_d)

            # ---------------- attention ----------------
            def scores_exp(mch, sk):
                """Row-packed score matmuls for heads (2mch, 2mch+1) vs
                key tile sk, then exp into per-sk E^T tiles."""
                ets = []
                for hh in range(2):
                    p0, p1 = hh * 64, hh * 64 + 64
                    ps = psA.tile([128, SQ], f32, tag="a")
                    for n in range(SQ // 512):
                        nc.tensor.matmul(
                            out=ps[:, n * 512:(n + 1) * 512],
                            lhsT=kT_s[p0:p1, mch, sk * 128:(sk + 1) * 128],
                            rhs=qT_s[p0:p1, mch, n * 512:(n + 1) * 512],
                            start=True, stop=True)
                    et = etp.tile([128, SQ], bf16, tag="et")
                    nc.scalar.activation(out=et, in_=ps, func=AF.Exp,
                                         scale=0.125)
                    ets.append(et)
                return ets

            def av(mch, sk, ets, pOs):
                for hh in range(2):
                    h = 2 * mch + hh
                    for n in range(2):
                        nc.tensor.matmul(
                            out=pOs[hh][:, n, :],
                            lhsT=v_aug[:, sk, h, :],
                            rhs=ets[hh][:, n * 512:(n + 1) * 512],
                            start=(sk == 0), stop=(sk == SK // 128 - 1))

            def normalize(mch, pOs):
                for hh in range(2):
                    pO = pOs[hh]
                    rs = dvp.tile([1, SQ], f32, tag="rs", bufs=1)
                    nc.vector.tensor_copy(
                        out=rs, in_=pO[64:65, :, :].rearrange(
                            "p a b -> p (a b)"))
                    nc.vector.reciprocal_approx_fast(out=rs, in_=rs)
                    rb = dvp.tile([HD, SQ], f32, tag="rb", bufs=1)
                    nc.gpsimd.partition_broadcast(out_ap=rb, in_ap=rs)
                    nc.vector.tensor_mul(
                        out=attn_oT[hh * HD:hh * HD + HD, mch, :],
                        in0=pO[0:64, :, :].rearrange("p a b -> p (a b)"),
                        in1=rb)

            for mch in range(H // 2):
                pOs = [psB.tile([65, 2, 512], f32, tag="b",
                                name=f"pO{mch}_{hh}") for hh in range(2)]
                if mch == 0:
                    # V proj rides inside pair 0's window; A@V lags so its
                    # psum pool slots only open after V proj vacates psA.
                    pend = []
                    for sk in range(SK // 128):
                        vproj_sk(sk)
                        pend.append(scores_exp(0, sk))
                        if sk >= LAG:
                            av(0, sk - LAG, pend[sk - LAG], pOs)
                            pend[sk - LAG] = None
                    for sk in range(SK // 128 - LAG, SK // 128):
                        av(0, sk, pend[sk], pOs)
                else:
                    for sk in range(SK // 128):
                        ets = scores_exp(mch, sk)
                        av(mch, sk, ets, pOs)
                normalize(mch, pOs)

            if dbg:
                for (dd, tt_src) in ((dq_d, qT_s), (dk_d, kT_s),
                                     (dv_d, v_aug), (da_d, attn_oT)):
                    fl = tt_src.rearrange("p a b c -> p (a b c)") if len(
                        tt_src.shape) == 4 else tt_src.rearrange(
                            "p a b -> p (a b)")
                    n_el = fl.shape[1]
                    for off in range(0, n_el, 512):
                        w = min(512, n_el - off)
                        tmp = lnw.tile([128, w], f32, tag="xc2", bufs=1)
                        nc.vector.tensor_copy(out=tmp, in_=fl[:, off:off + w])
                        nc.sync.dma_start(out=dd[:, off:off + w], in_=tmp)

            # ---------------- out-proj + gate + final LN ----------------
            if not triv_lno:
                lnog_b = const.tile([128, D], f32)
                nc.sync.dma_start(out=lnog_b, in_=bcast_rows(lnog_d, 128))
                lnob_b = const.tile([128, D], f32)
                nc.sync.dma_start(out=lnob_b, in_=bcast_rows(lnob_d, 128))
            for tt in range(SQ // 128):
                ps2 = psA.tile([128, 2, 512], f32, tag="a")
                stats = lnw.tile([128, 2, 6], f32, tag="st2")
                for n in range(2):
                    for kt in range(KT):
                        nc.tensor.matmul(
                            out=ps2[:, n, :],
                            lhsT=attn_oT[:, kt, tt * 128:(tt + 1) * 128],
                            rhs=wo_s[:, kt, n * 512:(n + 1) * 512],
                            start=(kt == 0),
                            stop=(kt == KT - 1 and triv_o))
                    if not triv_o:
                        nc.tensor.matmul(
                            out=ps2[:, n, :], lhsT=ones_row,
                            rhs=bo_s[:, n * 512:(n + 1) * 512],
                            start=False, stop=True)
                    nc.vector.bn_stats(out=stats[:, n, :], in_=ps2[:, n, :])
                mv = lnw.tile([128, 2], f32, tag="mv2")
                nc.vector.bn_aggr(out=mv, in_=stats)
                # LN(c*x) = (x-mean(x)) * c/sqrt(c^2 var(x)+eps) * g + b
                gc = gate_s[:, tt:tt + 1]
                gv = lnw.tile([128, 1], f32, tag="gv")
                nc.vector.tensor_mul(out=gv, in0=gc, in1=gc)
                nc.vector.tensor_mul(out=gv, in0=gv, in1=mv[:, 1:2])
                rstd = lnw.tile([128, 1], f32, tag="rs2")
                nc.scalar.activation(out=rstd, in_=gv, func=AF.Ln,
                                     bias=eps_t)
                nc.scalar.activation(out=rstd, in_=rstd, func=AF.Exp,
                                     scale=-0.5)
                sc = lnw.tile([128, 1], f32, tag="sc")
                nc.vector.tensor_mul(out=sc, in0=rstd, in1=gc)
                mb = lnw.tile([128, 1], f32, tag="mb")
                nc.vector.tensor_mul(out=mb, in0=mv[:, 0:1], in1=sc)
                nc.vector.tensor_scalar_mul(out=mb, in0=mb, scalar1=-1.0)
                xc = lnw.tile([128, D], f32, tag="xc2", bufs=1)
                nc.vector.tensor_scalar(
                    out=xc, in0=ps2.rearrange("p a b -> p (a b)"),
                    scalar1=sc, scalar2=mb, op0=OP.mult, op1=OP.add)
                if triv_lno:
                    res = xc
                else:
                    res = lnw.tile([128, D], f32, tag="res")
                    nc.vector.tensor_mul(out=res, in0=xc, in1=lnog_b)
                    nc.vector.tensor_add(out=res, in0=res, in1=lnob_b)
                nc.sync.dma_start(
                    out=out_d[tt * 128:(tt + 1) * 128, :], in_=res)

    nc.compile()
    return nc


def _maybe_enable_trace():
    """Install the axon NTFF profile hook if tracing was requested."""
    if not os.environ.get("BASS_KERNEL_TRACE"):
        return False
    try:
        import sys
        import types
        import antenv
        if "antenv.axon_hooks" not in sys.modules:
            mod = types.ModuleType("antenv.axon_hooks")
            mod._hook = None
            mod.set_axon_ntff_profile_hook = lambda h: setattr(mod, "_hook", h)
            mod.get_axon_ntff_profile_hook = lambda: mod._hook
            sys.modules["antenv.axon_hooks"] = mod
            antenv.axon_hooks = mod
        from antenv.axon_hooks import get_axon_ntff_profile_hook
        if get_axon_ntff_profile_hook() is None:
            from trn_agent_boot.trn_boot import _ntff_profile_via_ctypes
            from antenv.axon_hooks import set_axon_ntff_profile_hook
            set_axon_ntff_profile_hook(
                _ntff_profile_via_ctypes("/opt/axon/libaxon_pjrt.so"))
        return True
    except Exception:
        return False


def kernel(**inputs):
    import ml_dtypes
    from concourse import bass_utils

    bf16 = ml_dtypes.bfloat16
    f = lambda k: np.asarray(inputs[k], dtype=np.float32)

    # ---- host-side folding of LN gains/betas into projections ----
    g_q, b_q = f("ln_q_g"), f("ln_q_b")
    g_kv, b_kv = f("ln_kv_g"), f("ln_kv_b")
    Wq, Wk, Wv, Wo = f("Wq"), f("Wk"), f("Wv"), f("Wo")
    Wg = f("Wg").reshape(D, 1)
    Wqf = g_q[:, None] * Wq
    Wkf = g_kv[:, None] * Wk
    Wvf = g_kv[:, None] * Wv
    Wgf = g_q[:, None] * Wg
    bqf = b_q @ Wq + f("bq")
    bkf = b_kv @ Wk + f("bk")
    bvf = b_kv @ Wv + f("bv")
    bgf = float((b_q @ Wg).reshape(()))
    bof = f("bo")

    def wlay(W):  # [D, D] -> [128, KT*D] bf16 (partition = in-dim % 128)
        return np.ascontiguousarray(
            W.reshape(KT, 128, D).transpose(1, 0, 2).reshape(128, KT * D)
        ).astype(bf16)

    def bcol(b):  # [D] -> [128, MT] f32 per-partition columns
        return np.ascontiguousarray(b.reshape(MT, 128).T)

    triv_b = not (bqf.any() or bkf.any() or bgf)
    triv_v = not bvf.any()
    triv_o = not bof.any()
    triv_lno = (not f("ln_o_b").any()) and bool(np.all(f("ln_o_g") == 1.0))
    key = ("nc", triv_b, triv_v, triv_o, triv_lno)
    if key not in _CACHE:
        _CACHE[key] = _build(triv_b, triv_v, triv_o, triv_lno)
    nc = _CACHE[key]

    shared = {
        "wq": wlay(Wqf), "wk": wlay(Wkf), "wv": wlay(Wvf), "wo": wlay(Wo),
        "wg": np.ascontiguousarray(Wgf.reshape(KT, 128).T).astype(bf16),
        "bqc": bcol(bqf), "bkc": bcol(bkf),
        "bvr": bvf.reshape(1, D).astype(bf16),
        "bor": bof.reshape(1, D).astype(bf16),
        "bgs": np.array([[bgf]], dtype=np.float32).astype(bf16),
        "lnog": f("ln_o_g").reshape(1, D),
        "lnob": f("ln_o_b").reshape(1, D),
    }
    query = f("query").astype(bf16)
    keyt = f("key").astype(bf16)
    value = f("value").astype(bf16)
    in_maps = []
    for c in range(N_CORES):
        b, hh = c // 2, c % 2
        in_maps.append({
            "xq": np.ascontiguousarray(query[b, hh * SQ:(hh + 1) * SQ, :]),
            "xk": np.ascontiguousarray(keyt[b]),
            "xv": np.ascontiguousarray(value[b]),
            **shared,
        })

    trace = _maybe_enable_trace()
    kw = {}
    if trace:
        kw = dict(trace=True, trace_cores=[0])
    res = bass_utils.run_bass_kernel_spmd(
        nc, in_maps, core_ids=list(range(N_CORES)), **kw)
    if trace:
        _CACHE["exec_time_ns"] = res.exec_time_ns
        _CACHE["trace_path"] = (res.instructions_and_trace[1]
                                if res.instructions_and_trace else None)

    out = np.empty((B, S, D), dtype=np.float32)
    for c in range(N_CORES):
        b, hh = c // 2, c % 2
        out[b, hh * SQ:(hh + 1) * SQ, :] = res.results[c]["out"]
    return out


# revision 17
# speedup vs baseline: 1.0535x; 1.0535x over previous
"""EnhancedMultiHeadAttention on 8 TRN2 NeuronCores.

Sharding: core c handles batch b=c//2 and query-row half h=c%2.
Each core computes the full attention for its 1024 query rows against its
batch's full 2048 keys/values. Outputs are disjoint slices of the full
[4, 2048, 1024] result, assembled on the host.

Host-side prep: LN gain/beta are folded into the q/k/v/gate projection
weights and biases in numpy (W' = diag(g) @ W, b' = beta @ W + b); weights
are shipped to the device in bf16 already laid out as [128, KT, D]
(partition = input-dim within contraction tile). Activations ship as bf16.

Device kernel structure (bf16 matmuls, f32 softmax/LN):
  - LayerNorm in token-major layout; normalized bf16 tiles are transposed
    via DMA-XBAR (128x128 tiles) straight into contraction-tile layout --
    no PE transposes, no PSUM staging.
  - ACT table sets are phase-grouped (sqrt for every LN rstd in the ramp,
    exp for gate+attention, sqrt again for the final LN) so the table RAM
    is loaded only a few times.  The V path needs no ACT at all during
    attention: its LN stats/rstd are precomputed in the ramp, the mean is
    subtracted on DVE, and the rstd rides the PSUM->SBUF copy of the
    projection as a per-partition (= per-token) scalar multiply.
  - K proj is chunk-major, Q proj m-chunk-major, so pair 0's scores unlock
    as early as possible.  V proj is interleaved into pair 0's attention
    window (its PSUM tiles rotate through the score pool slots).
  - Scores are computed transposed [Sk, Sq] with the two heads of an
    m-chunk row-packed as concurrent K=64 matmuls in disjoint row-group
    halves of the PE array (the layout already places head 2m in
    partitions 0-63 and head 2m+1 in 64-127).
  - exp (no max subtraction -- scores ~N(0,1)) writes per-sk bf16 E^T
    tiles into a small ring; A@V lags the exps by 2 key tiles so PSUM
    hand-offs never starve ACT.  A@V uses v (with a ones column appended
    per head) as the stationary operand so the softmax denominator falls
    out as psum row 64; denominators are copied to SBUF on DVE, inverted
    with reciprocal_approx_fast, broadcast via gpsimd, applied on DVE
    writing attn_out^T directly in the out-proj layout.
  - out-proj + gate folded into the final LN, big per-element passes on
    DVE instead of ACT.
"""

import os
import numpy as np

D = 1024
H = 16
HD = 64
S = 2048
B = 4
SQ = 1024  # query rows per core
SK = 2048  # kv rows per core
KT = D // 128  # contraction tiles
MT = D // 128  # output chunks
N_CORES = 8
EPS = 1e-5
LAG = 2  # A@V lag (in sk tiles) behind exp

_CACHE = {}


def _build(triv_b=False, triv_v=False, triv_o=False, triv_lno=False):
    """triv_b: folded q/k/gate biases all zero; triv_v: folded v bias zero;
    triv_o: bo zero; triv_lno: final LN gain==1 and beta==0."""
    from contextlib import ExitStack

    import concourse.bacc as bacc
    import concourse.bass as bass
    import concourse.mybir as mybir
    import concourse.tile as tile

    f32 = mybir.dt.float32
    bf16 = mybir.dt.bfloat16
    AF = mybir.ActivationFunctionType
    OP = mybir.AluOpType

    nc = bacc.Bacc("TRN2", target_bir_lowering=False, debug=False,
                   num_devices=N_CORES)

    xq = nc.dram_tensor("xq", [SQ, D], bf16, kind="ExternalInput").ap()
    xk = nc.dram_tensor("xk", [SK, D], bf16, kind="ExternalInput").ap()
    xv = nc.dram_tensor("xv", [SK, D], bf16, kind="ExternalInput").ap()
    # weights pre-folded + pre-laid-out on host: [128, KT*D] bf16
    wq_d = nc.dram_tensor("wq", [128, KT * D], bf16, kind="ExternalInput").ap()
    wk_d = nc.dram_tensor("wk", [128, KT * D], bf16, kind="ExternalInput").ap()
    wv_d = nc.dram_tensor("wv", [128, KT * D], bf16, kind="ExternalInput").ap()
    wo_d = nc.dram_tensor("wo", [128, KT * D], bf16, kind="ExternalInput").ap()
    wg_d = nc.dram_tensor("wg", [128, KT], bf16, kind="ExternalInput").ap()
    # biases (already folded on host)
    bq_d = nc.dram_tensor("bqc", [128, MT], f32, kind="ExternalInput").ap()
    bk_d = nc.dram_tensor("bkc", [128, MT], f32, kind="ExternalInput").ap()
    bv_d = nc.dram_tensor("bvr", [1, D], bf16, kind="ExternalInput").ap()
    bo_d = nc.dram_tensor("bor", [1, D], bf16, kind="ExternalInput").ap()
    bg_d = nc.dram_tensor("bgs", [1, 1], bf16, kind="ExternalInput").ap()
    lnog_d = nc.dram_tensor("lnog", [1, D], f32, kind="ExternalInput").ap()
    lnob_d = nc.dram_tensor("lnob", [1, D], f32, kind="ExternalInput").ap()
    out_d = nc.dram_tensor("out", [SQ, D], f32, kind="ExternalOutput").ap()
    dbg = os.environ.get("KDBG")
    if dbg:
        dq_d = nc.dram_tensor("dbg_qT", [128, MT * SQ], f32,
                              kind="ExternalOutput").ap()
        dk_d = nc.dram_tensor("dbg_kT", [128, MT * SK], f32,
                              kind="ExternalOutput").ap()
        dv_d = nc.dram_tensor("dbg_v", [128, (SK // 128) * H * (HD + 1)], f32,
                              kind="ExternalOutput").ap()
        da_d = nc.dram_tensor("dbg_aT", [128, KT * SQ], f32,
                              kind="ExternalOutput").ap()

    def bcast_rows(ap2d, p):
        return bass.AP(tensor=ap2d.tensor, offset=ap2d.offset,
                       ap=[[0, p]] + list(ap2d.ap[1:]))

    with tile.TileContext(nc) as tc:
        with ExitStack() as ctx:
            const = ctx.enter_context(tc.tile_pool(name="const", bufs=1))
            main = ctx.enter_context(tc.tile_pool(name="main", bufs=1))
            wpl = ctx.enter_context(tc.tile_pool(name="wpl", bufs=2))
            xnt = ctx.enter_context(tc.tile_pool(name="xnt", bufs=2))
            lnw = ctx.enter_context(tc.tile_pool(name="lnw", bufs=3))
            etp = ctx.enter_context(tc.tile_pool(name="etp", bufs=6))
            dvp = ctx.enter_context(tc.tile_pool(name="dvp", bufs=2))
            psA = ctx.enter_context(
                tc.tile_pool(name="psA", bufs=2, space="PSUM"))
            psB = ctx.enter_context(
                tc.tile_pool(name="psB", bufs=2, space="PSUM"))

            eps_t = const.tile([128, 1], f32)
            nc.vector.memset(eps_t, EPS)
            ones_row = const.tile([1, 128], bf16)
            nc.vector.memset(ones_row, 1.0)

            # persistent per-core intermediates
            kT_s = main.tile([128, MT, SK], bf16)
            qT_s = main.tile([128, MT, SQ], bf16)
            v_aug = main.tile([128, SK // 128, H, HD + 1], bf16)
            attn_oT = main.tile([128, KT, SQ], bf16)
            gate_s = main.tile([128, SQ // 128], f32)
            mu_v = main.tile([128, SK // 128], f32)
            rstd_v = main.tile([128, SK // 128], f32)
            var_v = main.tile([128, SK // 128], f32)
            nc.vector.memset(v_aug[:, :, :, HD:HD + 1], 1.0)

            if not triv_b:
                bqc = const.tile([128, MT], f32)
                nc.sync.dma_start(out=bqc, in_=bq_d)
                bkc = const.tile([128, MT], f32)
                nc.sync.dma_start(out=bkc, in_=bk_d)
                bg_s = const.tile([1, 1], bf16)
                nc.sync.dma_start(out=bg_s, in_=bg_d)
            if not triv_v:
                bvr = const.tile([1, D], bf16)
                nc.sync.dma_start(out=bvr, in_=bv_d)
                bvb = const.tile([128, D], bf16)
                nc.gpsimd.partition_broadcast(out_ap=bvb, in_ap=bvr)
            if not triv_o:
                bo_s = const.tile([1, D], bf16)
                nc.sync.dma_start(out=bo_s, in_=bo_d)
            wg_s = const.tile([128, KT], bf16)
            nc.sync.dma_start(out=wg_s, in_=wg_d)

            def ln_stats(xt):
                """bn stats of a [128, D] tile -> mv [128, 2] (mean, var)."""
                xt3 = xt.rearrange("p (s f) -> p s f", s=2)
                stats = lnw.tile([128, 2, 6], f32, tag="st")
                nc.vector.bn_stats(out=stats[:, 0, :], in_=xt3[:, 0, :])
                nc.vector.bn_stats(out=stats[:, 1, :], in_=xt3[:, 1, :])
                mv = lnw.tile([128, 2], f32, tag="mv")
                nc.vector.bn_aggr(out=mv, in_=stats)
                return mv

            def ln_tile(x_dram, t, chunk, col):
                """Full LN of token tile t (rstd on ACT Sqrt + DVE recip),
                then DMA-XBAR transpose into chunk[:, c, col:col+128]."""
                xt = lnw.tile([128, D], bf16, tag="x", bufs=2)
                nc.sync.dma_start(out=xt,
                                  in_=x_dram[t * 128:(t + 1) * 128, :])
                mv = ln_stats(xt)
                rstd = lnw.tile([128, 1], f32, tag="rs")
                nc.scalar.activation(out=rstd, in_=mv[:, 1:2],
                                     func=AF.Sqrt, bias=eps_t)
                nc.vector.reciprocal(out=rstd, in_=rstd)
                xc = lnw.tile([128, D], bf16, tag="xc", bufs=2)
                nc.vector.tensor_scalar(
                    out=xc, in0=xt, scalar1=mv[:, 0:1], scalar2=rstd,
                    op0=OP.subtract, op1=OP.mult)
                for c in range(KT):
                    nc.sync.dma_start(
                        out=chunk[:, c, col:col + 128],
                        in_=xc[:, c * 128:(c + 1) * 128], transpose=True)

            def proj_chunk(Ws, bcol, chunk, cc, dst, ms):
                """dst[:, m, cc*512:+512] = (W'^T xn^T + b') for m in ms."""
                for m in ms:
                    ps = psA.tile([128, 512], f32, tag="a")
                    for kt in range(KT):
                        nc.tensor.matmul(
                            out=ps,
                            lhsT=Ws[:, kt, m * 128:(m + 1) * 128],
                            rhs=chunk[:, kt, :],
                            start=(kt == 0), stop=(kt == KT - 1))
                    if bcol is None:
                        nc.vector.tensor_copy(
                            out=dst[:, m, cc * 512:(cc + 1) * 512], in_=ps)
                    else:
                        nc.vector.tensor_scalar_add(
                            out=dst[:, m, cc * 512:(cc + 1) * 512], in0=ps,
                            scalar1=bcol[:, m:m + 1])

            # ---------------- K path (chunk-major) ----------------
            wk_s = wpl.tile([128, KT, D], bf16, tag="w", name="wk")
            nc.sync.dma_start(out=wk_s.rearrange("p a b -> p (a b)"),
                              in_=wk_d)
            for cc in range(4):
                knc = xnt.tile([128, KT, 512], bf16, tag="kn",
                               name=f"knT{cc}")
                for t in range(4 * cc, 4 * cc + 4):
                    ln_tile(xk, t, knc, (t % 4) * 128)
                proj_chunk(wk_s, None if triv_b else bkc, knc, cc, kT_s,
                           range(MT))

            # ---------------- Q path (m-major) ----------------
            wq_s = wpl.tile([128, KT, D], bf16, tag="w", name="wq")
            nc.sync.dma_start(out=wq_s.rearrange("p a b -> p (a b)"),
                              in_=wq_d)
            qnc = []
            for cc in range(2):
                c = xnt.tile([128, KT, 512], bf16, tag="qn", name=f"qnT{cc}")
                qnc.append(c)
                for t in range(4 * cc, 4 * cc + 4):
                    ln_tile(xq, t, c, (t % 4) * 128)
            for m in range(MT):
                for cc in range(2):
                    proj_chunk(wq_s, None if triv_b else bqc, qnc[cc], cc,
                               qT_s, [m])

            # ---------------- V stats (ramp; apply happens later) --------
            for t in range(SK // 128):
                xt = lnw.tile([128, D], bf16, tag="x", bufs=2)
                nc.sync.dma_start(out=xt, in_=xv[t * 128:(t + 1) * 128, :])
                mv = ln_stats(xt)
                nc.vector.tensor_copy(out=mu_v[:, t:t + 1], in_=mv[:, 0:1])
                nc.vector.tensor_copy(out=var_v[:, t:t + 1], in_=mv[:, 1:2])
            nc.scalar.activation(out=rstd_v, in_=var_v, func=AF.Sqrt,
                                 bias=eps_t)
            nc.vector.reciprocal(out=rstd_v, in_=rstd_v)

            # ---------------- gate (exp-based sigmoid) ----------------
            for tt in range(SQ // 128):
                gps = psB.tile([128, 1], f32, tag="b")
                for kt in range(KT):
                    nc.tensor.matmul(
                        out=gps,
                        lhsT=qnc[tt // 4][:, kt, (tt % 4) * 128:
                                          (tt % 4 + 1) * 128],
                        rhs=wg_s[:, kt:kt + 1],
                        start=(kt == 0), stop=(kt == KT - 1 and triv_b))
                if not triv_b:
                    nc.tensor.matmul(out=gps, lhsT=ones_row, rhs=bg_s,
                                     start=False, stop=True)
                nc.scalar.activation(out=gate_s[:, tt:tt + 1], in_=gps,
                                     func=AF.Exp, scale=-1.0)
            nc.vector.tensor_scalar_add(out=gate_s, in0=gate_s, scalar1=1.0)
            nc.vector.reciprocal_approx_fast(out=gate_s, in_=gate_s)

            # ---------------- V weights (proj is interleaved) -------
            wv_s = wpl.tile([128, KT, D], bf16, tag="w", name="wv")
            nc.sync.dma_start(out=wv_s.rearrange("p a b -> p (a b)"),
                              in_=wv_d)

            def vproj_sk(sk):
                """v_aug[:, sk, :, :HD] = rstd_v * ((xv-mu) W_v') + b_v."""
                xt = lnw.tile([128, D], bf16, tag="x", bufs=2)
                nc.sync.dma_start(out=xt, in_=xv[sk * 128:(sk + 1) * 128, :])
                xc = lnw.tile([128, D], bf16, tag="xc", bufs=2)
                nc.vector.tensor_scalar_sub(out=xc, in0=xt,
                                            scalar1=mu_v[:, sk:sk + 1])
                vnc = xnt.tile([128, KT, 128], bf16, tag="vn", bufs=3)
                for c in range(KT):
                    nc.sync.dma_start(out=vnc[:, c, :],
                                      in_=xc[:, c * 128:(c + 1) * 128],
                                      transpose=True)
                for n in range(2):
                    ps = psA.tile([128, 512], f32, tag="a")
                    for kt in range(KT):
                        nc.tensor.matmul(
                            out=ps,
                            lhsT=vnc[:, kt, :],
                            rhs=wv_s[:, kt, n * 512:(n + 1) * 512],
                            start=(kt == 0), stop=(kt == KT - 1))
                    nc.vector.tensor_scalar_mul(
                        out=v_aug[:, sk, n * 8:(n + 1) * 8, 0:HD],
                        in0=ps.rearrange("p (h d) -> p h d", h=8),
                        scalar1=rstd_v[:, sk:sk + 1])
                    if not triv_v:
                        nc.vector.tensor_add(
                            out=v_aug[:, sk, n * 8:(n + 1) * 8, 0:HD],
                            in0=v_aug[:, sk, n * 8:(n + 1) * 8, 0:HD],
                            in1=bvb[:, n * 512:(n + 1) * 512].rearrange(
                                "p (h d) -> p h d", h=8))

            # load Wo into the slot the w rotation frees
            wo_s = wpl.tile([128, KT, D], bf16, tag="w", name="wo")
            nc.sync.dma_start(out=wo_s.rearrange("p a b -> p (a b)"),
                              in_=wo_d)

            # ---------------- attention ----------------
            def scores_exp(mch, sk):
                """Row-packed score matmuls for heads (2mch, 2mch+1) vs
                key tile sk, then exp into per-sk E^T tiles."""
                ets = []
                for hh in range(2):
                    p0, p1 = hh * 64, hh * 64 + 64
                    ps = psA.tile([128, SQ], f32, tag="a")
                    for n in range(SQ // 512):
                        nc.tensor.matmul(
                            out=ps[:, n * 512:(n + 1) * 512],
                            lhsT=kT_s[p0:p1, mch, sk * 128:(sk + 1) * 128],
                            rhs=qT_s[p0:p1, mch, n * 512:(n + 1) * 512],
                            start=True, stop=True)
                    et = etp.tile([128, SQ], bf16, tag="et")
                    nc.scalar.activation(out=et, in_=ps, func=AF.Exp,
                                         scale=0.125)
                    ets.append(et)
                return ets

            def av(mch, sk, ets, pOs):
                for hh in range(2):
                    h = 2 * mch + hh
                    for n in range(2):
                        nc.tensor.matmul(
                            out=pOs[hh][:, n, :],
                            lhsT=v_aug[:, sk, h, :],
                            rhs=ets[hh][:, n * 512:(n + 1) * 512],
                            start=(sk == 0), stop=(sk == SK // 128 - 1))

            def normalize(mch, pOs):
                for hh in range(2):
                    pO = pOs[hh]
                    rs = dvp.tile([1, SQ], f32, tag="rs", bufs=1)
                    nc.vector.tensor_copy(
                        out=rs, in_=pO[64:65, :, :].rearrange(
                            "p a b -> p (a b)"))
                    nc.vector.reciprocal_approx_fast(out=rs, in_=rs)
                    rb = dvp.tile([HD, SQ], f32, tag="rb", bufs=1)
                    nc.gpsimd.partition_broadcast(out_ap=rb, in_ap=rs)
                    nc.vector.tensor_mul(
                        out=attn_oT[hh * HD:hh * HD + HD, mch, :],
                        in0=pO[0:64, :, :].rearrange("p a b -> p (a b)"),
                        in1=rb)

            for mch in range(H // 2):
                pOs = [psB.tile([65, 2, 512], f32, tag="b",
                                name=f"pO{mch}_{hh}") for hh in range(2)]
                pend = {}
                for sk in range(SK // 128):
                    if mch == 0:
                        vproj_sk(sk)
                    pend[sk] = scores_exp(mch, sk)
                    if sk >= LAG:
                        av(mch, sk - LAG, pend.pop(sk - LAG), pOs)
                for sk in range(SK // 128 - LAG, SK // 128):
                    av(mch, sk, pend.pop(sk), pOs)
                normalize(mch, pOs)

            if dbg:
                for (dd, tt_src) in ((dq_d, qT_s), (dk_d, kT_s),
                                     (dv_d, v_aug), (da_d, attn_oT)):
                    fl = tt_src.rearrange("p a b c -> p (a b c)") if len(
                        tt_src.shape) == 4 else tt_src.rearrange(
                            "p a b -> p (a b)")
                    n_el = fl.shape[1]
                    for off in range(0, n_el, 512):
                        w = min(512, n_el - off)
                        tmp = lnw.tile([128, w], f32, tag="xc2", bufs=1)
                        nc.vector.tensor_copy(out=tmp, in_=fl[:, off:off + w])
                        nc.sync.dma_start(out=dd[:, off:off + w], in_=tmp)

            # ---------------- out-proj + gate + final LN ----------------
            if not triv_lno:
                lnog_b = const.tile([128, D], f32)
                nc.sync.dma_start(out=lnog_b, in_=bcast_rows(lnog_d, 128))
                lnob_b = const.tile([128, D], f32)
                nc.sync.dma_start(out=lnob_b, in_=bcast_rows(lnob_d, 128))
            for tt in range(SQ // 128):
                ps2 = psA.tile([128, 2, 512], f32, tag="a")
                stats = lnw.tile([128, 2, 6], f32, tag="st2")
                for n in range(2):
                    for kt in range(KT):
                        nc.tensor.matmul(
                            out=ps2[:, n, :],
                            lhsT=attn_oT[:, kt, tt * 128:(tt + 1) * 128],
                            rhs=wo_s[:, kt, n * 512:(n + 1) * 512],
                            start=(kt == 0),
                            stop=(kt == KT - 1 and triv_o))
                    if not triv_o:
                        nc.tensor.matmul(
                            out=ps2[:, n, :], lhsT=ones_row,
                            rhs=bo_s[:, n * 512:(n + 1) * 512],
                            start=False, stop=True)
                    nc.vector.bn_stats(out=stats[:, n, :], in_=ps2[:, n, :])
                mv = lnw.tile([128, 2], f32, tag="mv2")
                nc.vector.bn_aggr(out=mv, in_=stats)
                # LN(c*x) = (x-mean(x)) * c/sqrt(c^2 var(x)+eps) * g + b
                gc = gate_s[:, tt:tt + 1]
                gv = lnw.tile([128, 1], f32, tag="gv")
                nc.vector.tensor_mul(out=gv, in0=gc, in1=gc)
                nc.vector.tensor_mul(out=gv, in0=gv, in1=mv[:, 1:2])
                rstd = lnw.tile([128, 1], f32, tag="rs2")
                nc.scalar.activation(out=rstd, in_=gv, func=AF.Sqrt,
                                     bias=eps_t)
                nc.vector.reciprocal(out=rstd, in_=rstd)
                sc = lnw.tile([128, 1], f32, tag="sc")
                nc.vector.tensor_mul(out=sc, in0=rstd, in1=gc)
                mb = lnw.tile([128, 1], f32, tag="mb")
                nc.vector.tensor_mul(out=mb, in0=mv[:, 0:1], in1=sc)
                nc.vector.tensor_scalar_mul(out=mb, in0=mb, scalar1=-1.0)
                xc = lnw.tile([128, D], f32, tag="xc2", bufs=1)
                nc.vector.tensor_scalar(
                    out=xc, in0=ps2.rearrange("p a b -> p (a b)"),
                    scalar1=sc, scalar2=mb, op0=OP.mult, op1=OP.add)
                if triv_lno:
                    res = xc
                else:
                    res = lnw.tile([128, D], f32, tag="res")
                    nc.vector.tensor_mul(out=res, in0=xc, in1=lnog_b)
                    nc.vector.tensor_add(out=res, in0=res, in1=lnob_b)
                nc.sync.dma_start(
                    out=out_d[tt * 128:(tt + 1) * 128, :], in_=res)

    nc.compile()
    return nc


def _maybe_enable_trace():
    """Install the axon NTFF profile hook if tracing was requested."""
    if not os.environ.get("BASS_KERNEL_TRACE"):
        return False
    try:
        import sys
        import types
        import antenv
        if "antenv.axon_hooks" not in sys.modules:
            mod = types.ModuleType("antenv.axon_hooks")
            mod._hook = None
            mod.set_axon_ntff_profile_hook = lambda h: setattr(mod, "_hook", h)
            mod.get_axon_ntff_profile_hook = lambda: mod._hook
            sys.modules["antenv.axon_hooks"] = mod
            antenv.axon_hooks = mod
        from antenv.axon_hooks import get_axon_ntff_profile_hook
        if get_axon_ntff_profile_hook() is None:
            from trn_agent_boot.trn_boot import _ntff_profile_via_ctypes
            from antenv.axon_hooks import set_axon_ntff_profile_hook
            set_axon_ntff_profile_hook(
                _ntff_profile_via_ctypes("/opt/axon/libaxon_pjrt.so"))
        return True
    except Exception:
        return False


def kernel(**inputs):
    import ml_dtypes
    from concourse import bass_utils

    bf16 = ml_dtypes.bfloat16
    f = lambda k: np.asarray(inputs[k], dtype=np.float32)

    # ---- host-side folding of LN gains/betas into projections ----
    g_q, b_q = f("ln_q_g"), f("ln_q_b")
    g_kv, b_kv = f("ln_kv_g"), f("ln_kv_b")
    Wq, Wk, Wv, Wo = f("Wq"), f("Wk"), f("Wv"), f("Wo")
    Wg = f("Wg").reshape(D, 1)
    Wqf = g_q[:, None] * Wq
    Wkf = g_kv[:, None] * Wk
    Wvf = g_kv[:, None] * Wv
    Wgf = g_q[:, None] * Wg
    bqf = b_q @ Wq + f("bq")
    bkf = b_kv @ Wk + f("bk")
    bvf = b_kv @ Wv + f("bv")
    bgf = float((b_q @ Wg).reshape(()))
    bof = f("bo")

    def wlay(W):  # [D, D] -> [128, KT*D] bf16 (partition = in-dim % 128)
        return np.ascontiguousarray(
            W.reshape(KT, 128, D).transpose(1, 0, 2).reshape(128, KT * D)
        ).astype(bf16)

    def bcol(b):  # [D] -> [128, MT] f32 per-partition columns
        return np.ascontiguousarray(b.reshape(MT, 128).T)

    triv_b = not (bqf.any() or bkf.any() or bgf)
    triv_v = not bvf.any()
    triv_o = not bof.any()
    triv_lno = (not f("ln_o_b").any()) and bool(np.all(f("ln_o_g") == 1.0))
    key = ("nc", triv_b, triv_v, triv_o, triv_lno)
    if key not in _CACHE:
        _CACHE[key] = _build(triv_b, triv_v, triv_o, triv_lno)
    nc = _CACHE[key]

    shared = {
        "wq": wlay(Wqf), "wk": wlay(Wkf), "wv": wlay(Wvf), "wo": wlay(Wo),
        "wg": np.ascontiguousarray(Wgf.reshape(KT, 128).T).astype(bf16),
        "bqc": bcol(bqf), "bkc": bcol(bkf),
        "bvr": bvf.reshape(1, D).astype(bf16),
        "bor": bof.reshape(1, D).astype(bf16),
        "bgs": np.array([[bgf]], dtype=np.float32).astype(bf16),
        "lnog": f("ln_o_g").reshape(1, D),
        "lnob": f("ln_o_b").reshape(1, D),
    }
    query = f("query").astype(bf16)
    keyt = f("key").astype(bf16)
    value = f("value").astype(bf16)
    in_maps = []
    for c in range(N_CORES):
        b, hh = c // 2, c % 2
        in_maps.append({
            "xq": np.ascontiguousarray(query[b, hh * SQ:(hh + 1) * SQ, :]),
            "xk": np.ascontiguousarray(keyt[b]),
            "xv": np.ascontiguousarray(value[b]),
            **shared,
        })

    trace = _maybe_enable_trace()
    kw = {}
    if trace:
        kw = dict(trace=True, trace_cores=[0])
    res = bass_utils.run_bass_kernel_spmd(
        nc, in_maps, core_ids=list(range(N_CORES)), **kw)
    if trace:
        _CACHE["exec_time_ns"] = res.exec_time_ns
        _CACHE["trace_path"] = (res.instructions_and_trace[1]
                                if res.instructions_and_trace else None)

    out = np.empty((B, S, D), dtype=np.float32)
    for c in range(N_CORES):
        b, hh = c // 2, c % 2
        out[b, hh * SQ:(hh + 1) * SQ, :] = res.results[c]["out"]
    return out


# revision 19
# speedup vs baseline: 1.6699x; 1.5852x over previous
"""EnhancedMultiHeadAttention on 8 TRN2 NeuronCores.

Sharding: core c handles batch b=c//2 and query-row half h=c%2.
Each core computes the full attention for its 1024 query rows against its
batch's full 2048 keys/values. Outputs are disjoint slices of the full
[4, 2048, 1024] result, assembled on the host.

Host-side prep: LN gain/beta are folded into the q/k/v/gate projection
weights and biases in numpy (W' = diag(g) @ W, b' = beta @ W + b); weights
are shipped to the device in bf16 already laid out as [128, KT, D]
(partition = input-dim within contraction tile). Activations ship as bf16.

Device kernel structure (bf16 matmuls, f32 softmax/LN):
  - LayerNorm in token-major layout; normalized bf16 tiles are transposed
    via DMA-XBAR (128x128 tiles) straight into contraction-tile layout --
    no PE transposes, no PSUM staging.
  - ACT table sets are phase-grouped (sqrt for every LN rstd in the ramp,
    exp for gate+attention, sqrt again for the final LN) so the table RAM
    is loaded only a few times.  The V path needs no ACT at all during
    attention: its LN stats/rstd are precomputed in the ramp, the mean is
    subtracted on DVE, and the rstd rides the PSUM->SBUF copy of the
    projection as a per-partition (= per-token) scalar multiply.
  - K proj is chunk-major, Q proj m-chunk-major, so pair 0's scores unlock
    as early as possible.  V proj is interleaved into pair 0's attention
    window (its PSUM tiles rotate through the score pool slots).
  - Scores are computed transposed [Sk, Sq] with the two heads of an
    m-chunk row-packed as concurrent K=64 matmuls in disjoint row-group
    halves of the PE array (the layout already places head 2m in
    partitions 0-63 and head 2m+1 in 64-127).
  - exp (no max subtraction -- scores ~N(0,1)) writes per-sk bf16 E^T
    tiles into a small ring; A@V lags the exps by 2 key tiles so PSUM
    hand-offs never starve ACT.  A@V uses v (with a ones column appended
    per head) as the stationary operand so the softmax denominator falls
    out as psum row 64; denominators are copied to SBUF on DVE, inverted
    with reciprocal_approx_fast, broadcast via gpsimd, applied on DVE
    writing attn_out^T directly in the out-proj layout.
  - out-proj + gate folded into the final LN, big per-element passes on
    DVE instead of ACT.
"""

import os
import numpy as np

D = 1024
H = 16
HD = 64
S = 2048
B = 4
SQ = 1024  # query rows per core
SK = 2048  # kv rows per core
KT = D // 128  # contraction tiles
MT = D // 128  # output chunks
N_CORES = 8
EPS = 1e-5
LAG = 2  # A@V lag (in sk tiles) behind exp

_CACHE = {}


def _build(triv_b=False, triv_v=False, triv_o=False, triv_lno=False):
    """triv_b: folded q/k/gate biases all zero; triv_v: folded v bias zero;
    triv_o: bo zero; triv_lno: final LN gain==1 and beta==0."""
    from contextlib import ExitStack

    import concourse.bacc as bacc
    import concourse.bass as bass
    import concourse.mybir as mybir
    import concourse.tile as tile
    from concourse.masks import make_identity

    f32 = mybir.dt.float32
    bf16 = mybir.dt.bfloat16
    AF = mybir.ActivationFunctionType
    OP = mybir.AluOpType

    nc = bacc.Bacc("TRN2", target_bir_lowering=False, debug=False,
                   num_devices=N_CORES)

    xq = nc.dram_tensor("xq", [SQ, D], bf16, kind="ExternalInput").ap()
    xk = nc.dram_tensor("xk", [SK, D], bf16, kind="ExternalInput").ap()
    xv = nc.dram_tensor("xv", [SK, D], bf16, kind="ExternalInput").ap()
    # weights pre-folded + pre-laid-out on host: [128, KT*D] bf16
    wq_d = nc.dram_tensor("wq", [128, KT * D], bf16, kind="ExternalInput").ap()
    wk_d = nc.dram_tensor("wk", [128, KT * D], bf16, kind="ExternalInput").ap()
    wv_d = nc.dram_tensor("wv", [128, KT * D], bf16, kind="ExternalInput").ap()
    wo_d = nc.dram_tensor("wo", [128, KT * D], bf16, kind="ExternalInput").ap()
    wg_d = nc.dram_tensor("wg", [128, KT], bf16, kind="ExternalInput").ap()
    # biases (already folded on host)
    bq_d = nc.dram_tensor("bqc", [128, MT], f32, kind="ExternalInput").ap()
    bk_d = nc.dram_tensor("bkc", [128, MT], f32, kind="ExternalInput").ap()
    bv_d = nc.dram_tensor("bvr", [1, D], bf16, kind="ExternalInput").ap()
    bo_d = nc.dram_tensor("bor", [1, D], bf16, kind="ExternalInput").ap()
    bg_d = nc.dram_tensor("bgs", [1, 1], bf16, kind="ExternalInput").ap()
    lnog_d = nc.dram_tensor("lnog", [1, D], f32, kind="ExternalInput").ap()
    lnob_d = nc.dram_tensor("lnob", [1, D], f32, kind="ExternalInput").ap()
    out_d = nc.dram_tensor("out", [SQ, D], f32, kind="ExternalOutput").ap()
    dbg = os.environ.get("KDBG")
    if dbg:
        dq_d = nc.dram_tensor("dbg_qT", [128, MT * SQ], f32,
                              kind="ExternalOutput").ap()
        dk_d = nc.dram_tensor("dbg_kT", [128, MT * SK], f32,
                              kind="ExternalOutput").ap()
        dv_d = nc.dram_tensor("dbg_v", [128, (SK // 128) * H * (HD + 1)], f32,
                              kind="ExternalOutput").ap()
        da_d = nc.dram_tensor("dbg_aT", [128, KT * SQ], f32,
                              kind="ExternalOutput").ap()

    def bcast_rows(ap2d, p):
        return bass.AP(tensor=ap2d.tensor, offset=ap2d.offset,
                       ap=[[0, p]] + list(ap2d.ap[1:]))

    with tile.TileContext(nc) as tc:
        with ExitStack() as ctx:
            const = ctx.enter_context(tc.tile_pool(name="const", bufs=1))
            main = ctx.enter_context(tc.tile_pool(name="main", bufs=1))
            wpl = ctx.enter_context(tc.tile_pool(name="wpl", bufs=2))
            xnt = ctx.enter_context(tc.tile_pool(name="xnt", bufs=2))
            lnw = ctx.enter_context(tc.tile_pool(name="lnw", bufs=3))
            etp = ctx.enter_context(tc.tile_pool(name="etp", bufs=6))
            dvp = ctx.enter_context(tc.tile_pool(name="dvp", bufs=2))
            psA = ctx.enter_context(
                tc.tile_pool(name="psA", bufs=2, space="PSUM"))
            psB = ctx.enter_context(
                tc.tile_pool(name="psB", bufs=2, space="PSUM"))

            eps_t = const.tile([128, 1], f32)
            nc.vector.memset(eps_t, EPS)
            ones_row = const.tile([1, 128], bf16)
            nc.vector.memset(ones_row, 1.0)
            identity = const.tile([128, 128], bf16)
            make_identity(nc, identity)

            # persistent per-core intermediates
            kT_s = main.tile([128, MT, SK], bf16)
            qT_s = main.tile([128, MT, SQ], bf16)
            v_aug = main.tile([128, SK // 128, H, HD + 1], bf16)
            attn_oT = main.tile([128, KT, SQ], bf16)
            gate_s = main.tile([128, SQ // 128], f32)
            mu_v = main.tile([128, SK // 128], f32)
            rstd_v = main.tile([128, SK // 128], f32)
            var_v = main.tile([128, SK // 128], f32)
            nc.vector.memset(v_aug[:, :, :, HD:HD + 1], 1.0)

            if not triv_b:
                bqc = const.tile([128, MT], f32)
                nc.sync.dma_start(out=bqc, in_=bq_d)
                bkc = const.tile([128, MT], f32)
                nc.sync.dma_start(out=bkc, in_=bk_d)
                bg_s = const.tile([1, 1], bf16)
                nc.sync.dma_start(out=bg_s, in_=bg_d)
            if not triv_v:
                bvr = const.tile([1, D], bf16)
                nc.sync.dma_start(out=bvr, in_=bv_d)
                bvb = const.tile([128, D], bf16)
                nc.gpsimd.partition_broadcast(out_ap=bvb, in_ap=bvr)
            if not triv_o:
                bo_s = const.tile([1, D], bf16)
                nc.sync.dma_start(out=bo_s, in_=bo_d)
            wg_s = const.tile([128, KT], bf16)
            nc.sync.dma_start(out=wg_s, in_=wg_d)

            def ln_stats(xt):
                """bn stats of a [128, D] tile -> mv [128, 2] (mean, var)."""
                xt3 = xt.rearrange("p (s f) -> p s f", s=2)
                stats = lnw.tile([128, 2, 6], f32, tag="st")
                nc.vector.bn_stats(out=stats[:, 0, :], in_=xt3[:, 0, :])
                nc.vector.bn_stats(out=stats[:, 1, :], in_=xt3[:, 1, :])
                mv = lnw.tile([128, 2], f32, tag="mv")
                nc.vector.bn_aggr(out=mv, in_=stats)
                return mv

            def ln_tile(x_dram, t, chunk, col):
                """Full LN of token tile t (rstd on ACT Sqrt + DVE recip),
                then DMA-XBAR transpose into chunk[:, c, col:col+128]."""
                xt = lnw.tile([128, D], bf16, tag="x", bufs=2)
                nc.sync.dma_start(out=xt,
                                  in_=x_dram[t * 128:(t + 1) * 128, :])
                mv = ln_stats(xt)
                rstd = lnw.tile([128, 1], f32, tag="rs")
                nc.scalar.activation(out=rstd, in_=mv[:, 1:2],
                                     func=AF.Sqrt, bias=eps_t)
                nc.vector.reciprocal(out=rstd, in_=rstd)
                xc = lnw.tile([128, D], bf16, tag="xc", bufs=2)
                nc.vector.tensor_scalar(
                    out=xc, in0=xt, scalar1=mv[:, 0:1], scalar2=rstd,
                    op0=OP.subtract, op1=OP.mult)
                pt = psA.tile([128, KT, 128], bf16, tag="a")
                for c in range(KT):
                    nc.tensor.transpose(
                        out=pt[:, c, :], in_=xc[:, c * 128:(c + 1) * 128],
                        identity=identity)
                nc.vector.tensor_copy(out=chunk[:, :, col:col + 128], in_=pt)

            def proj_chunk(Ws, bcol, chunk, cc, dst, ms):
                """dst[:, m, cc*512:+512] = (W'^T xn^T + b') for m in ms."""
                for m in ms:
                    ps = psA.tile([128, 512], f32, tag="a")
                    for kt in range(KT):
                        nc.tensor.matmul(
                            out=ps,
                            lhsT=Ws[:, kt, m * 128:(m + 1) * 128],
                            rhs=chunk[:, kt, :],
                            start=(kt == 0), stop=(kt == KT - 1))
                    if bcol is None:
                        nc.vector.tensor_copy(
                            out=dst[:, m, cc * 512:(cc + 1) * 512], in_=ps)
                    else:
                        nc.vector.tensor_scalar_add(
                            out=dst[:, m, cc * 512:(cc + 1) * 512], in0=ps,
                            scalar1=bcol[:, m:m + 1])

            # ---------------- K path (chunk-major) ----------------
            wk_s = wpl.tile([128, KT, D], bf16, tag="w", name="wk")
            nc.sync.dma_start(out=wk_s.rearrange("p a b -> p (a b)"),
                              in_=wk_d)
            for cc in range(4):
                knc = xnt.tile([128, KT, 512], bf16, tag="kn",
                               name=f"knT{cc}")
                for t in range(4 * cc, 4 * cc + 4):
                    ln_tile(xk, t, knc, (t % 4) * 128)
                proj_chunk(wk_s, None if triv_b else bkc, knc, cc, kT_s,
                           range(MT))

            # ---------------- Q path (m-major) ----------------
            wq_s = wpl.tile([128, KT, D], bf16, tag="w", name="wq")
            nc.sync.dma_start(out=wq_s.rearrange("p a b -> p (a b)"),
                              in_=wq_d)
            qnc = []
            for cc in range(2):
                c = xnt.tile([128, KT, 512], bf16, tag="qn", name=f"qnT{cc}")
                qnc.append(c)
                for t in range(4 * cc, 4 * cc + 4):
                    ln_tile(xq, t, c, (t % 4) * 128)
            for m in range(MT):
                for cc in range(2):
                    proj_chunk(wq_s, None if triv_b else bqc, qnc[cc], cc,
                               qT_s, [m])

            # ---------------- V stats (ramp; apply happens later) --------
            for t in range(SK // 128):
                xt = lnw.tile([128, D], bf16, tag="x", bufs=2)
                nc.sync.dma_start(out=xt, in_=xv[t * 128:(t + 1) * 128, :])
                mv = ln_stats(xt)
                nc.vector.tensor_copy(out=mu_v[:, t:t + 1], in_=mv[:, 0:1])
                nc.vector.tensor_copy(out=var_v[:, t:t + 1], in_=mv[:, 1:2])
            nc.scalar.activation(out=rstd_v, in_=var_v, func=AF.Sqrt,
                                 bias=eps_t)
            nc.vector.reciprocal(out=rstd_v, in_=rstd_v)

            # ---------------- gate (exp-based sigmoid) ----------------
            for tt in range(SQ // 128):
                gps = psB.tile([128, 1], f32, tag="b")
                for kt in range(KT):
                    nc.tensor.matmul(
                        out=gps,
                        lhsT=qnc[tt // 4][:, kt, (tt % 4) * 128:
                                          (tt % 4 + 1) * 128],
                        rhs=wg_s[:, kt:kt + 1],
                        start=(kt == 0), stop=(kt == KT - 1 and triv_b))
                if not triv_b:
                    nc.tensor.matmul(out=gps, lhsT=ones_row, rhs=bg_s,
                                     start=False, stop=True)
                nc.scalar.activation(out=gate_s[:, tt:tt + 1], in_=gps,
                                     func=AF.Exp, scale=-1.0)
            nc.vector.tensor_scalar_add(out=gate_s, in0=gate_s, scalar1=1.0)
            nc.vector.reciprocal_approx_fast(out=gate_s, in_=gate_s)

            # ---------------- V weights (proj is interleaved) -------
            wv_s = wpl.tile([128, KT, D], bf16, tag="w", name="wv")
            nc.sync.dma_start(out=wv_s.rearrange("p a b -> p (a b)"),
                              in_=wv_d)

            def vproj_sk(sk):
                """v_aug[:, sk, :, :HD] = rstd_v * ((xv-mu) W_v') + b_v."""
                xt = lnw.tile([128, D], bf16, tag="x", bufs=2)
                nc.sync.dma_start(out=xt, in_=xv[sk * 128:(sk + 1) * 128, :])
                xc = lnw.tile([128, D], bf16, tag="xc", bufs=2)
                nc.vector.tensor_scalar_sub(out=xc, in0=xt,
                                            scalar1=mu_v[:, sk:sk + 1])
                pt = psA.tile([128, KT, 128], bf16, tag="a")
                for c in range(KT):
                    nc.tensor.transpose(
                        out=pt[:, c, :], in_=xc[:, c * 128:(c + 1) * 128],
                        identity=identity)
                vnc = xnt.tile([128, KT, 128], bf16, tag="vn", bufs=3)
                nc.vector.tensor_copy(out=vnc, in_=pt)
                for n in range(2):
                    ps = psA.tile([128, 512], f32, tag="a")
                    for kt in range(KT):
                        nc.tensor.matmul(
                            out=ps,
                            lhsT=vnc[:, kt, :],
                            rhs=wv_s[:, kt, n * 512:(n + 1) * 512],
                            start=(kt == 0), stop=(kt == KT - 1))
                    nc.vector.tensor_scalar_mul(
                        out=v_aug[:, sk, n * 8:(n + 1) * 8, 0:HD],
                        in0=ps.rearrange("p (h d) -> p h d", h=8),
                        scalar1=rstd_v[:, sk:sk + 1])
                    if not triv_v:
                        nc.vector.tensor_add(
                            out=v_aug[:, sk, n * 8:(n + 1) * 8, 0:HD],
                            in0=v_aug[:, sk, n * 8:(n + 1) * 8, 0:HD],
                            in1=bvb[:, n * 512:(n + 1) * 512].rearrange(
                                "p (h d) -> p h d", h=8))

            # load Wo into the slot the w rotation frees
            wo_s = wpl.tile([128, KT, D], bf16, tag="w", name="wo")
            nc.sync.dma_start(out=wo_s.rearrange("p a b -> p (a b)"),
                              in_=wo_d)

            # ---------------- attention ----------------
            def scores_exp(mch, sk):
                """Row-packed score matmuls for heads (2mch, 2mch+1) vs
                key tile sk, then exp into per-sk E^T tiles."""
                ets = []
                for hh in range(2):
                    p0, p1 = hh * 64, hh * 64 + 64
                    ps = psA.tile([128, SQ], f32, tag="a")
                    for n in range(SQ // 512):
                        nc.tensor.matmul(
                            out=ps[:, n * 512:(n + 1) * 512],
                            lhsT=kT_s[p0:p1, mch, sk * 128:(sk + 1) * 128],
                            rhs=qT_s[p0:p1, mch, n * 512:(n + 1) * 512],
                            start=True, stop=True)
                    et = etp.tile([128, SQ], bf16, tag="et")
                    nc.scalar.activation(out=et, in_=ps, func=AF.Exp,
                                         scale=0.125)
                    ets.append(et)
                return ets

            def av(mch, sk, ets, pOs):
                for hh in range(2):
                    h = 2 * mch + hh
                    for n in range(2):
                        nc.tensor.matmul(
                            out=pOs[hh][:, n, :],
                            lhsT=v_aug[:, sk, h, :],
                            rhs=ets[hh][:, n * 512:(n + 1) * 512],
                            start=(sk == 0), stop=(sk == SK // 128 - 1))

            def normalize(mch, pOs):
                for hh in range(2):
                    pO = pOs[hh]
                    rs = dvp.tile([1, SQ], f32, tag="rs", bufs=1)
                    nc.vector.tensor_copy(
                        out=rs, in_=pO[64:65, :, :].rearrange(
                            "p a b -> p (a b)"))
                    nc.vector.reciprocal_approx_fast(out=rs, in_=rs)
                    rb = dvp.tile([HD, SQ], f32, tag="rb", bufs=1)
                    nc.gpsimd.partition_broadcast(out_ap=rb, in_ap=rs)
                    nc.vector.tensor_mul(
                        out=attn_oT[hh * HD:hh * HD + HD, mch, :],
                        in0=pO[0:64, :, :].rearrange("p a b -> p (a b)"),
                        in1=rb)

            for mch in range(H // 2):
                pOs = [psB.tile([65, 2, 512], f32, tag="b",
                                name=f"pO{mch}_{hh}") for hh in range(2)]
                pend = {}
                for sk in range(SK // 128):
                    if mch == 0:
                        vproj_sk(sk)
                    pend[sk] = scores_exp(mch, sk)
                    if sk >= LAG:
                        av(mch, sk - LAG, pend.pop(sk - LAG), pOs)
                for sk in range(SK // 128 - LAG, SK // 128):
                    av(mch, sk, pend.pop(sk), pOs)
                normalize(mch, pOs)

            if dbg:
                for (dd, tt_src) in ((dq_d, qT_s), (dk_d, kT_s),
                                     (dv_d, v_aug), (da_d, attn_oT)):
                    fl = tt_src.rearrange("p a b c -> p (a b c)") if len(
                        tt_src.shape) == 4 else tt_src.rearrange(
                            "p a b -> p (a b)")
                    n_el = fl.shape[1]
                    for off in range(0, n_el, 512):
                        w = min(512, n_el - off)
                        tmp = lnw.tile([128, w], f32, tag="xc2", bufs=1)
                        nc.vector.tensor_copy(out=tmp, in_=fl[:, off:off + w])
                        nc.sync.dma_start(out=dd[:, off:off + w], in_=tmp)

            # ---------------- out-proj + gate + final LN ----------------
            if not triv_lno:
                lnog_b = const.tile([128, D], f32)
                nc.sync.dma_start(out=lnog_b, in_=bcast_rows(lnog_d, 128))
                lnob_b = const.tile([128, D], f32)
                nc.sync.dma_start(out=lnob_b, in_=bcast_rows(lnob_d, 128))
            for tt in range(SQ // 128):
                ps2 = psA.tile([128, 2, 512], f32, tag="a")
                stats = lnw.tile([128, 2, 6], f32, tag="st2")
                for n in range(2):
                    for kt in range(KT):
                        nc.tensor.matmul(
                            out=ps2[:, n, :],
                            lhsT=attn_oT[:, kt, tt * 128:(tt + 1) * 128],
                            rhs=wo_s[:, kt, n * 512:(n + 1) * 512],
                            start=(kt == 0),
                            stop=(kt == KT - 1 and triv_o))
                    if not triv_o:
                        nc.tensor.matmul(
                            out=ps2[:, n, :], lhsT=ones_row,
                            rhs=bo_s[:, n * 512:(n + 1) * 512],
                            start=False, stop=True)
                    nc.vector.bn_stats(out=stats[:, n, :], in_=ps2[:, n, :])
                mv = lnw.tile([128, 2], f32, tag="mv2")
                nc.vector.bn_aggr(out=mv, in_=stats)
                # LN(c*x) = (x-mean(x)) * c/sqrt(c^2 var(x)+eps) * g + b
                gc = gate_s[:, tt:tt + 1]
                gv = lnw.tile([128, 1], f32, tag="gv")
                nc.vector.tensor_mul(out=gv, in0=gc, in1=gc)
                nc.vector.tensor_mul(out=gv, in0=gv, in1=mv[:, 1:2])
                rstd = lnw.tile([128, 1], f32, tag="rs2")
                nc.scalar.activation(out=rstd, in_=gv, func=AF.Sqrt,
                                     bias=eps_t)
                nc.vector.reciprocal(out=rstd, in_=rstd)
                sc = lnw.tile([128, 1], f32, tag="sc")
                nc.vector.tensor_mul(out=sc, in0=rstd, in1=gc)
                mb = lnw.tile([128, 1], f32, tag="mb")
                nc.vector.tensor_mul(out=mb, in0=mv[:, 0:1], in1=sc)
                nc.vector.tensor_scalar_mul(out=mb, in0=mb, scalar1=-1.0)
                xc = lnw.tile([128, D], f32, tag="xc2", bufs=1)
                nc.vector.tensor_scalar(
                    out=xc, in0=ps2.rearrange("p a b -> p (a b)"),
                    scalar1=sc, scalar2=mb, op0=OP.mult, op1=OP.add)
                if triv_lno:
                    res = xc
                else:
                    res = lnw.tile([128, D], f32, tag="res")
                    nc.vector.tensor_mul(out=res, in0=xc, in1=lnog_b)
                    nc.vector.tensor_add(out=res, in0=res, in1=lnob_b)
                nc.sync.dma_start(
                    out=out_d[tt * 128:(tt + 1) * 128, :], in_=res)

    nc.compile()
    return nc


def _maybe_enable_trace():
    """Install the axon NTFF profile hook if tracing was requested."""
    if not os.environ.get("BASS_KERNEL_TRACE"):
        return False
    try:
        import sys
        import types
        import antenv
        if "antenv.axon_hooks" not in sys.modules:
            mod = types.ModuleType("antenv.axon_hooks")
            mod._hook = None
            mod.set_axon_ntff_profile_hook = lambda h: setattr(mod, "_hook", h)
            mod.get_axon_ntff_profile_hook = lambda: mod._hook
            sys.modules["antenv.axon_hooks"] = mod
            antenv.axon_hooks = mod
        from antenv.axon_hooks import get_axon_ntff_profile_hook
        if get_axon_ntff_profile_hook() is None:
            from trn_agent_boot.trn_boot import _ntff_profile_via_ctypes
            from antenv.axon_hooks import set_axon_ntff_profile_hook
            set_axon_ntff_profile_hook(
                _ntff_profile_via_ctypes("/opt/axon/libaxon_pjrt.so"))
        return True
    except Exception:
        return False


def kernel(**inputs):
    import ml_dtypes
    from concourse import bass_utils

    bf16 = ml_dtypes.bfloat16
    f = lambda k: np.asarray(inputs[k], dtype=np.float32)

    # ---- host-side folding of LN gains/betas into projections ----
    g_q, b_q = f("ln_q_g"), f("ln_q_b")
    g_kv, b_kv = f("ln_kv_g"), f("ln_kv_b")
    Wq, Wk, Wv, Wo = f("Wq"), f("Wk"), f("Wv"), f("Wo")
    Wg = f("Wg").reshape(D, 1)
    Wqf = g_q[:, None] * Wq
    Wkf = g_kv[:, None] * Wk
    Wvf = g_kv[:, None] * Wv
    Wgf = g_q[:, None] * Wg
    bqf = b_q @ Wq + f("bq")
    bkf = b_kv @ Wk + f("bk")
    bvf = b_kv @ Wv + f("bv")
    bgf = float((b_q @ Wg).reshape(()))
    bof = f("bo")

    def wlay(W):  # [D, D] -> [128, KT*D] bf16 (partition = in-dim % 128)
        return np.ascontiguousarray(
            W.reshape(KT, 128, D).transpose(1, 0, 2).reshape(128, KT * D)
        ).astype(bf16)

    def bcol(b):  # [D] -> [128, MT] f32 per-partition columns
        return np.ascontiguousarray(b.reshape(MT, 128).T)

    triv_b = not (bqf.any() or bkf.any() or bgf)
    triv_v = not bvf.any()
    triv_o = not bof.any()
    triv_lno = (not f("ln_o_b").any()) and bool(np.all(f("ln_o_g") == 1.0))
    key = ("nc", triv_b, triv_v, triv_o, triv_lno)
    if key not in _CACHE:
        _CACHE[key] = _build(triv_b, triv_v, triv_o, triv_lno)
    nc = _CACHE[key]

    shared = {
        "wq": wlay(Wqf), "wk": wlay(Wkf), "wv": wlay(Wvf), "wo": wlay(Wo),
        "wg": np.ascontiguousarray(Wgf.reshape(KT, 128).T).astype(bf16),
        "bqc": bcol(bqf), "bkc": bcol(bkf),
        "bvr": bvf.reshape(1, D).astype(bf16),
        "bor": bof.reshape(1, D).astype(bf16),
        "bgs": np.array([[bgf]], dtype=np.float32).astype(bf16),
        "lnog": f("ln_o_g").reshape(1, D),
        "lnob": f("ln_o_b").reshape(1, D),
    }
    query = f("query").astype(bf16)
    keyt = f("key").astype(bf16)
    value = f("value").astype(bf16)
    in_maps = []
    for c in range(N_CORES):
        b, hh = c // 2, c % 2
        in_maps.append({
            "xq": np.ascontiguousarray(query[b, hh * SQ:(hh + 1) * SQ, :]),
            "xk": np.ascontiguousarray(keyt[b]),
            "xv": np.ascontiguousarray(value[b]),
            **shared,
        })

    trace = _maybe_enable_trace()
    kw = {}
    if trace:
        kw = dict(trace=True, trace_cores=[0])
    res = bass_utils.run_bass_kernel_spmd(
        nc, in_maps, core_ids=list(range(N_CORES)), **kw)
    if trace:
        _CACHE["exec_time_ns"] = res.exec_time_ns
        _CACHE["trace_path"] = (res.instructions_and_trace[1]
                                if res.instructions_and_trace else None)

    out = np.empty((B, S, D), dtype=np.float32)
    for c in range(N_CORES):
        b, hh = c // 2, c % 2
        out[b, hh * SQ:(hh + 1) * SQ, :] = res.results[c]["out"]
    return out


# revision 20
# speedup vs baseline: 1.6726x; 1.0016x over previous
"""EnhancedMultiHeadAttention on 8 TRN2 NeuronCores.

Sharding: core c handles batch b=c//2 and query-row half h=c%2.
Each core computes the full attention for its 1024 query rows against its
batch's full 2048 keys/values. Outputs are disjoint slices of the full
[4, 2048, 1024] result, assembled on the host.

Host-side prep: LN gain/beta are folded into the q/k/v/gate projection
weights and biases in numpy (W' = diag(g) @ W, b' = beta @ W + b); weights
are shipped to the device in bf16 already laid out as [128, KT, D]
(partition = input-dim within contraction tile). Activations ship as bf16.

Device kernel structure (bf16 matmuls, f32 softmax/LN):
  - LayerNorm in token-major layout; normalized bf16 tiles are transposed
    via DMA-XBAR (128x128 tiles) straight into contraction-tile layout --
    no PE transposes, no PSUM staging.
  - ACT table sets are phase-grouped (sqrt for every LN rstd in the ramp,
    exp for gate+attention, sqrt again for the final LN) so the table RAM
    is loaded only a few times.  The V path needs no ACT at all during
    attention: its LN stats/rstd are precomputed in the ramp, the mean is
    subtracted on DVE, and the rstd rides the PSUM->SBUF copy of the
    projection as a per-partition (= per-token) scalar multiply.
  - K proj is chunk-major, Q proj m-chunk-major, so pair 0's scores unlock
    as early as possible.  V proj is interleaved into pair 0's attention
    window (its PSUM tiles rotate through the score pool slots).
  - Scores are computed transposed [Sk, Sq] with the two heads of an
    m-chunk row-packed as concurrent K=64 matmuls in disjoint row-group
    halves of the PE array (the layout already places head 2m in
    partitions 0-63 and head 2m+1 in 64-127).
  - exp (no max subtraction -- scores ~N(0,1)) writes per-sk bf16 E^T
    tiles into a small ring; A@V lags the exps by 2 key tiles so PSUM
    hand-offs never starve ACT.  A@V uses v (with a ones column appended
    per head) as the stationary operand so the softmax denominator falls
    out as psum row 64; denominators are copied to SBUF on DVE, inverted
    with reciprocal_approx_fast, broadcast via gpsimd, applied on DVE
    writing attn_out^T directly in the out-proj layout.
  - out-proj + gate folded into the final LN, big per-element passes on
    DVE instead of ACT.
"""

import os
import numpy as np

D = 1024
H = 16
HD = 64
S = 2048
B = 4
SQ = 1024  # query rows per core
SK = 2048  # kv rows per core
KT = D // 128  # contraction tiles
MT = D // 128  # output chunks
N_CORES = 8
EPS = 1e-5
LAG = 2  # A@V lag (in sk tiles) behind exp

_CACHE = {}


def _build(triv_b=False, triv_v=False, triv_o=False, triv_lno=False):
    """triv_b: folded q/k/gate biases all zero; triv_v: folded v bias zero;
    triv_o: bo zero; triv_lno: final LN gain==1 and beta==0."""
    from contextlib import ExitStack

    import concourse.bacc as bacc
    import concourse.bass as bass
    import concourse.mybir as mybir
    import concourse.tile as tile
    from concourse.masks import make_identity

    f32 = mybir.dt.float32
    bf16 = mybir.dt.bfloat16
    AF = mybir.ActivationFunctionType
    OP = mybir.AluOpType

    nc = bacc.Bacc("TRN2", target_bir_lowering=False, debug=False,
                   num_devices=N_CORES)

    xq = nc.dram_tensor("xq", [SQ, D], bf16, kind="ExternalInput").ap()
    xk = nc.dram_tensor("xk", [SK, D], bf16, kind="ExternalInput").ap()
    xv = nc.dram_tensor("xv", [SK, D], bf16, kind="ExternalInput").ap()
    # weights pre-folded + pre-laid-out on host: [128, KT*D] bf16
    wq_d = nc.dram_tensor("wq", [128, KT * D], bf16, kind="ExternalInput").ap()
    wk_d = nc.dram_tensor("wk", [128, KT * D], bf16, kind="ExternalInput").ap()
    wv_d = nc.dram_tensor("wv", [128, KT * D], bf16, kind="ExternalInput").ap()
    wo_d = nc.dram_tensor("wo", [128, KT * D], bf16, kind="ExternalInput").ap()
    wg_d = nc.dram_tensor("wg", [128, KT], bf16, kind="ExternalInput").ap()
    # biases (already folded on host)
    bq_d = nc.dram_tensor("bqc", [128, MT], f32, kind="ExternalInput").ap()
    bk_d = nc.dram_tensor("bkc", [128, MT], f32, kind="ExternalInput").ap()
    bv_d = nc.dram_tensor("bvr", [1, D], bf16, kind="ExternalInput").ap()
    bo_d = nc.dram_tensor("bor", [1, D], bf16, kind="ExternalInput").ap()
    bg_d = nc.dram_tensor("bgs", [1, 1], bf16, kind="ExternalInput").ap()
    lnog_d = nc.dram_tensor("lnog", [1, D], f32, kind="ExternalInput").ap()
    lnob_d = nc.dram_tensor("lnob", [1, D], f32, kind="ExternalInput").ap()
    out_d = nc.dram_tensor("out", [SQ, D], f32, kind="ExternalOutput").ap()
    dbg = os.environ.get("KDBG")
    if dbg:
        dq_d = nc.dram_tensor("dbg_qT", [128, MT * SQ], f32,
                              kind="ExternalOutput").ap()
        dk_d = nc.dram_tensor("dbg_kT", [128, MT * SK], f32,
                              kind="ExternalOutput").ap()
        dv_d = nc.dram_tensor("dbg_v", [128, (SK // 128) * H * (HD + 1)], f32,
                              kind="ExternalOutput").ap()
        da_d = nc.dram_tensor("dbg_aT", [128, KT * SQ], f32,
                              kind="ExternalOutput").ap()

    def bcast_rows(ap2d, p):
        return bass.AP(tensor=ap2d.tensor, offset=ap2d.offset,
                       ap=[[0, p]] + list(ap2d.ap[1:]))

    with tile.TileContext(nc) as tc:
        with ExitStack() as ctx:
            const = ctx.enter_context(tc.tile_pool(name="const", bufs=1))
            main = ctx.enter_context(tc.tile_pool(name="main", bufs=1))
            wpl = ctx.enter_context(tc.tile_pool(name="wpl", bufs=2))
            xnt = ctx.enter_context(tc.tile_pool(name="xnt", bufs=2))
            lnw = ctx.enter_context(tc.tile_pool(name="lnw", bufs=3))
            etp = ctx.enter_context(tc.tile_pool(name="etp", bufs=5))
            dvp = ctx.enter_context(tc.tile_pool(name="dvp", bufs=2))
            psA = ctx.enter_context(
                tc.tile_pool(name="psA", bufs=2, space="PSUM"))
            psB = ctx.enter_context(
                tc.tile_pool(name="psB", bufs=2, space="PSUM"))

            eps_t = const.tile([128, 1], f32)
            nc.vector.memset(eps_t, EPS)
            ones_row = const.tile([1, 128], bf16)
            nc.vector.memset(ones_row, 1.0)
            identity = const.tile([128, 128], bf16)
            make_identity(nc, identity)

            # persistent per-core intermediates
            kT_s = main.tile([128, MT, SK], bf16)
            qT_s = main.tile([128, MT, SQ], bf16)
            v_aug = main.tile([128, SK // 128, H, HD + 1], bf16)
            attn_oT = main.tile([128, KT, SQ], bf16)
            gate_s = main.tile([128, SQ // 128], f32)
            mu_v = main.tile([128, SK // 128], f32)
            rstd_v = main.tile([128, SK // 128], f32)
            var_v = main.tile([128, SK // 128], f32)
            nc.vector.memset(v_aug[:, :, :, HD:HD + 1], 1.0)

            if not triv_b:
                bqc = const.tile([128, MT], f32)
                nc.sync.dma_start(out=bqc, in_=bq_d)
                bkc = const.tile([128, MT], f32)
                nc.sync.dma_start(out=bkc, in_=bk_d)
                bg_s = const.tile([1, 1], bf16)
                nc.sync.dma_start(out=bg_s, in_=bg_d)
            if not triv_v:
                bvr = const.tile([1, D], bf16)
                nc.sync.dma_start(out=bvr, in_=bv_d)
                bvb = const.tile([128, D], bf16)
                nc.gpsimd.partition_broadcast(out_ap=bvb, in_ap=bvr)
            if not triv_o:
                bo_s = const.tile([1, D], bf16)
                nc.sync.dma_start(out=bo_s, in_=bo_d)
            wg_s = const.tile([128, KT], bf16)
            nc.sync.dma_start(out=wg_s, in_=wg_d)

            def ln_stats(xt):
                """bn stats of a [128, D] tile -> mv [128, 2] (mean, var)."""
                xt3 = xt.rearrange("p (s f) -> p s f", s=2)
                stats = lnw.tile([128, 2, 6], f32, tag="st")
                nc.vector.bn_stats(out=stats[:, 0, :], in_=xt3[:, 0, :])
                nc.vector.bn_stats(out=stats[:, 1, :], in_=xt3[:, 1, :])
                mv = lnw.tile([128, 2], f32, tag="mv")
                nc.vector.bn_aggr(out=mv, in_=stats)
                return mv

            def ln_tile(x_dram, t, chunk, col):
                """Full LN of token tile t (rstd on ACT Sqrt + DVE recip),
                then DMA-XBAR transpose into chunk[:, c, col:col+128]."""
                xt = lnw.tile([128, D], bf16, tag="x", bufs=3)
                nc.sync.dma_start(out=xt,
                                  in_=x_dram[t * 128:(t + 1) * 128, :])
                mv = ln_stats(xt)
                rstd = lnw.tile([128, 1], f32, tag="rs")
                nc.scalar.activation(out=rstd, in_=mv[:, 1:2],
                                     func=AF.Sqrt, bias=eps_t)
                nc.vector.reciprocal(out=rstd, in_=rstd)
                xc = lnw.tile([128, D], bf16, tag="xc", bufs=3)
                nc.vector.tensor_scalar(
                    out=xc, in0=xt, scalar1=mv[:, 0:1], scalar2=rstd,
                    op0=OP.subtract, op1=OP.mult)
                pt = psA.tile([128, KT, 128], bf16, tag="a")
                for c in range(KT):
                    nc.tensor.transpose(
                        out=pt[:, c, :], in_=xc[:, c * 128:(c + 1) * 128],
                        identity=identity)
                nc.vector.tensor_copy(out=chunk[:, :, col:col + 128], in_=pt)

            def proj_chunk(Ws, bcol, chunk, cc, dst, ms):
                """dst[:, m, cc*512:+512] = (W'^T xn^T + b') for m in ms."""
                for m in ms:
                    ps = psA.tile([128, 512], f32, tag="a")
                    for kt in range(KT):
                        nc.tensor.matmul(
                            out=ps,
                            lhsT=Ws[:, kt, m * 128:(m + 1) * 128],
                            rhs=chunk[:, kt, :],
                            start=(kt == 0), stop=(kt == KT - 1))
                    if bcol is None:
                        nc.vector.tensor_copy(
                            out=dst[:, m, cc * 512:(cc + 1) * 512], in_=ps)
                    else:
                        nc.vector.tensor_scalar_add(
                            out=dst[:, m, cc * 512:(cc + 1) * 512], in0=ps,
                            scalar1=bcol[:, m:m + 1])

            # ---------------- K path (chunk-major) ----------------
            wk_s = wpl.tile([128, KT, D], bf16, tag="w", name="wk")
            nc.sync.dma_start(out=wk_s.rearrange("p a b -> p (a b)"),
                              in_=wk_d)
            for cc in range(4):
                knc = xnt.tile([128, KT, 512], bf16, tag="kn",
                               name=f"knT{cc}")
                for t in range(4 * cc, 4 * cc + 4):
                    ln_tile(xk, t, knc, (t % 4) * 128)
                proj_chunk(wk_s, None if triv_b else bkc, knc, cc, kT_s,
                           range(MT))

            # ---------------- Q path (m-major) ----------------
            wq_s = wpl.tile([128, KT, D], bf16, tag="w", name="wq")
            nc.sync.dma_start(out=wq_s.rearrange("p a b -> p (a b)"),
                              in_=wq_d)
            qnc = []
            for cc in range(2):
                c = xnt.tile([128, KT, 512], bf16, tag="qn", name=f"qnT{cc}")
                qnc.append(c)
                for t in range(4 * cc, 4 * cc + 4):
                    ln_tile(xq, t, c, (t % 4) * 128)
            for m in range(MT):
                for cc in range(2):
                    proj_chunk(wq_s, None if triv_b else bqc, qnc[cc], cc,
                               qT_s, [m])

            # ---------------- V stats (ramp; apply happens later) --------
            for t in range(SK // 128):
                xt = lnw.tile([128, D], bf16, tag="x", bufs=3)
                nc.sync.dma_start(out=xt, in_=xv[t * 128:(t + 1) * 128, :])
                mv = ln_stats(xt)
                nc.vector.tensor_copy(out=mu_v[:, t:t + 1], in_=mv[:, 0:1])
                nc.vector.tensor_copy(out=var_v[:, t:t + 1], in_=mv[:, 1:2])
            nc.scalar.activation(out=rstd_v, in_=var_v, func=AF.Sqrt,
                                 bias=eps_t)
            nc.vector.reciprocal(out=rstd_v, in_=rstd_v)

            # ---------------- gate (exp-based sigmoid) ----------------
            for tt in range(SQ // 128):
                gps = psB.tile([128, 1], f32, tag="b")
                for kt in range(KT):
                    nc.tensor.matmul(
                        out=gps,
                        lhsT=qnc[tt // 4][:, kt, (tt % 4) * 128:
                                          (tt % 4 + 1) * 128],
                        rhs=wg_s[:, kt:kt + 1],
                        start=(kt == 0), stop=(kt == KT - 1 and triv_b))
                if not triv_b:
                    nc.tensor.matmul(out=gps, lhsT=ones_row, rhs=bg_s,
                                     start=False, stop=True)
                nc.scalar.activation(out=gate_s[:, tt:tt + 1], in_=gps,
                                     func=AF.Exp, scale=-1.0)
            nc.vector.tensor_scalar_add(out=gate_s, in0=gate_s, scalar1=1.0)
            nc.vector.reciprocal_approx_fast(out=gate_s, in_=gate_s)

            # ---------------- V weights (proj is interleaved) -------
            wv_s = wpl.tile([128, KT, D], bf16, tag="w", name="wv")
            nc.sync.dma_start(out=wv_s.rearrange("p a b -> p (a b)"),
                              in_=wv_d)

            def vproj_sk(sk):
                """v_aug[:, sk, :, :HD] = rstd_v * ((xv-mu) W_v') + b_v."""
                xt = lnw.tile([128, D], bf16, tag="x", bufs=3)
                nc.sync.dma_start(out=xt, in_=xv[sk * 128:(sk + 1) * 128, :])
                xc = lnw.tile([128, D], bf16, tag="xc", bufs=3)
                nc.vector.tensor_scalar_sub(out=xc, in0=xt,
                                            scalar1=mu_v[:, sk:sk + 1])
                pt = psA.tile([128, KT, 128], bf16, tag="a")
                for c in range(KT):
                    nc.tensor.transpose(
                        out=pt[:, c, :], in_=xc[:, c * 128:(c + 1) * 128],
                        identity=identity)
                vnc = xnt.tile([128, KT, 128], bf16, tag="vn", bufs=3)
                nc.vector.tensor_copy(out=vnc, in_=pt)
                for n in range(2):
                    ps = psA.tile([128, 512], f32, tag="a")
                    for kt in range(KT):
                        nc.tensor.matmul(
                            out=ps,
                            lhsT=vnc[:, kt, :],
                            rhs=wv_s[:, kt, n * 512:(n + 1) * 512],
                            start=(kt == 0), stop=(kt == KT - 1))
                    nc.vector.tensor_scalar_mul(
                        out=v_aug[:, sk, n * 8:(n + 1) * 8, 0:HD],
                        in0=ps.rearrange("p (h d) -> p h d", h=8),
                        scalar1=rstd_v[:, sk:sk + 1])
                    if not triv_v:
                        nc.vector.tensor_add(
                            out=v_aug[:, sk, n * 8:(n + 1) * 8, 0:HD],
                            in0=v_aug[:, sk, n * 8:(n + 1) * 8, 0:HD],
                            in1=bvb[:, n * 512:(n + 1) * 512].rearrange(
                                "p (h d) -> p h d", h=8))

            # load Wo into the slot the w rotation frees
            wo_s = wpl.tile([128, KT, D], bf16, tag="w", name="wo")
            nc.sync.dma_start(out=wo_s.rearrange("p a b -> p (a b)"),
                              in_=wo_d)

            # ---------------- attention ----------------
            def scores_exp(mch, sk):
                """Row-packed score matmuls for heads (2mch, 2mch+1) vs
                key tile sk, then exp into per-sk E^T tiles."""
                ets = []
                for hh in range(2):
                    p0, p1 = hh * 64, hh * 64 + 64
                    ps = psA.tile([128, SQ], f32, tag="a")
                    for n in range(SQ // 512):
                        nc.tensor.matmul(
                            out=ps[:, n * 512:(n + 1) * 512],
                            lhsT=kT_s[p0:p1, mch, sk * 128:(sk + 1) * 128],
                            rhs=qT_s[p0:p1, mch, n * 512:(n + 1) * 512],
                            start=True, stop=True)
                    et = etp.tile([128, SQ], bf16, tag="et")
                    nc.scalar.activation(out=et, in_=ps, func=AF.Exp,
                                         scale=0.125)
                    ets.append(et)
                return ets

            def av(mch, sk, ets, pOs):
                for hh in range(2):
                    h = 2 * mch + hh
                    for n in range(2):
                        nc.tensor.matmul(
                            out=pOs[hh][:, n, :],
                            lhsT=v_aug[:, sk, h, :],
                            rhs=ets[hh][:, n * 512:(n + 1) * 512],
                            start=(sk == 0), stop=(sk == SK // 128 - 1))

            def normalize(mch, pOs):
                for hh in range(2):
                    pO = pOs[hh]
                    rs = dvp.tile([1, SQ], f32, tag="rs", bufs=1)
                    nc.vector.tensor_copy(
                        out=rs, in_=pO[64:65, :, :].rearrange(
                            "p a b -> p (a b)"))
                    nc.vector.reciprocal_approx_fast(out=rs, in_=rs)
                    rb = dvp.tile([HD, SQ], f32, tag="rb", bufs=1)
                    nc.gpsimd.partition_broadcast(out_ap=rb, in_ap=rs)
                    nc.vector.tensor_mul(
                        out=attn_oT[hh * HD:hh * HD + HD, mch, :],
                        in0=pO[0:64, :, :].rearrange("p a b -> p (a b)"),
                        in1=rb)

            for mch in range(H // 2):
                pOs = [psB.tile([65, 2, 512], f32, tag="b",
                                name=f"pO{mch}_{hh}") for hh in range(2)]
                pend = {}
                for sk in range(SK // 128):
                    if sk >= LAG:
                        av(mch, sk - LAG, pend.pop(sk - LAG), pOs)
                    if mch == 0:
                        vproj_sk(sk)
                    pend[sk] = scores_exp(mch, sk)
                for sk in range(SK // 128 - LAG, SK // 128):
                    av(mch, sk, pend.pop(sk), pOs)
                normalize(mch, pOs)

            if dbg:
                for (dd, tt_src) in ((dq_d, qT_s), (dk_d, kT_s),
                                     (dv_d, v_aug), (da_d, attn_oT)):
                    fl = tt_src.rearrange("p a b c -> p (a b c)") if len(
                        tt_src.shape) == 4 else tt_src.rearrange(
                            "p a b -> p (a b)")
                    n_el = fl.shape[1]
                    for off in range(0, n_el, 512):
                        w = min(512, n_el - off)
                        tmp = lnw.tile([128, w], f32, tag="xc2", bufs=1)
                        nc.vector.tensor_copy(out=tmp, in_=fl[:, off:off + w])
                        nc.sync.dma_start(out=dd[:, off:off + w], in_=tmp)

            # ---------------- out-proj + gate + final LN ----------------
            if not triv_lno:
                lnog_b = const.tile([128, D], f32)
                nc.sync.dma_start(out=lnog_b, in_=bcast_rows(lnog_d, 128))
                lnob_b = const.tile([128, D], f32)
                nc.sync.dma_start(out=lnob_b, in_=bcast_rows(lnob_d, 128))
            for tt in range(SQ // 128):
                ps2 = psA.tile([128, 2, 512], f32, tag="a")
                stats = lnw.tile([128, 2, 6], f32, tag="st2")
                for n in range(2):
                    for kt in range(KT):
                        nc.tensor.matmul(
                            out=ps2[:, n, :],
                            lhsT=attn_oT[:, kt, tt * 128:(tt + 1) * 128],
                            rhs=wo_s[:, kt, n * 512:(n + 1) * 512],
                            start=(kt == 0),
                            stop=(kt == KT - 1 and triv_o))
                    if not triv_o:
                        nc.tensor.matmul(
                            out=ps2[:, n, :], lhsT=ones_row,
                            rhs=bo_s[:, n * 512:(n + 1) * 512],
                            start=False, stop=True)
                    nc.vector.bn_stats(out=stats[:, n, :], in_=ps2[:, n, :])
                mv = lnw.tile([128, 2], f32, tag="mv2")
                nc.vector.bn_aggr(out=mv, in_=stats)
                # LN(c*x) = (x-mean(x)) * c/sqrt(c^2 var(x)+eps) * g + b
                gc = gate_s[:, tt:tt + 1]
                gv = lnw.tile([128, 1], f32, tag="gv")
                nc.vector.tensor_mul(out=gv, in0=gc, in1=gc)
                nc.vector.tensor_mul(out=gv, in0=gv, in1=mv[:, 1:2])
                rstd = lnw.tile([128, 1], f32, tag="rs2")
                nc.scalar.activation(out=rstd, in_=gv, func=AF.Sqrt,
                                     bias=eps_t)
                nc.vector.reciprocal(out=rstd, in_=rstd)
                sc = lnw.tile([128, 1], f32, tag="sc")
                nc.vector.tensor_mul(out=sc, in0=rstd, in1=gc)
                mb = lnw.tile([128, 1], f32, tag="mb")
                nc.vector.tensor_mul(out=mb, in0=mv[:, 0:1], in1=sc)
                nc.vector.tensor_scalar_mul(out=mb, in0=mb, scalar1=-1.0)
                xc = lnw.tile([128, D], f32, tag="xc2", bufs=1)
                nc.vector.tensor_scalar(
                    out=xc, in0=ps2.rearrange("p a b -> p (a b)"),
                    scalar1=sc, scalar2=mb, op0=OP.mult, op1=OP.add)
                if triv_lno:
                    res = xc
                else:
                    res = lnw.tile([128, D], f32, tag="res")
                    nc.vector.tensor_mul(out=res, in0=xc, in1=lnog_b)
                    nc.vector.tensor_add(out=res, in0=res, in1=lnob_b)
                nc.sync.dma_start(
                    out=out_d[tt * 128:(tt + 1) * 128, :], in_=res)

    nc.compile()
    return nc


def _maybe_enable_trace():
    """Install the axon NTFF profile hook if tracing was requested."""
    if not os.environ.get("BASS_KERNEL_TRACE"):
        return False
    try:
        import sys
        import types
        import antenv
        if "antenv.axon_hooks" not in sys.modules:
            mod = types.ModuleType("antenv.axon_hooks")
            mod._hook = None
            mod.set_axon_ntff_profile_hook = lambda h: setattr(mod, "_hook", h)
            mod.get_axon_ntff_profile_hook = lambda: mod._hook
            sys.modules["antenv.axon_hooks"] = mod
            antenv.axon_hooks = mod
        from antenv.axon_hooks import get_axon_ntff_profile_hook
        if get_axon_ntff_profile_hook() is None:
            from trn_agent_boot.trn_boot import _ntff_profile_via_ctypes
            from antenv.axon_hooks import set_axon_ntff_profile_hook
            set_axon_ntff_profile_hook(
                _ntff_profile_via_ctypes("/opt/axon/libaxon_pjrt.so"))
        return True
    except Exception:
        return False


def kernel(**inputs):
    import ml_dtypes
    from concourse import bass_utils

    bf16 = ml_dtypes.bfloat16
    f = lambda k: np.asarray(inputs[k], dtype=np.float32)

    # ---- host-side folding of LN gains/betas into projections ----
    g_q, b_q = f("ln_q_g"), f("ln_q_b")
    g_kv, b_kv = f("ln_kv_g"), f("ln_kv_b")
    Wq, Wk, Wv, Wo = f("Wq"), f("Wk"), f("Wv"), f("Wo")
    Wg = f("Wg").reshape(D, 1)
    Wqf = g_q[:, None] * Wq
    Wkf = g_kv[:, None] * Wk
    Wvf = g_kv[:, None] * Wv
    Wgf = g_q[:, None] * Wg
    bqf = b_q @ Wq + f("bq")
    bkf = b_kv @ Wk + f("bk")
    bvf = b_kv @ Wv + f("bv")
    bgf = float((b_q @ Wg).reshape(()))
    bof = f("bo")

    def wlay(W):  # [D, D] -> [128, KT*D] bf16 (partition = in-dim % 128)
        return np.ascontiguousarray(
            W.reshape(KT, 128, D).transpose(1, 0, 2).reshape(128, KT * D)
        ).astype(bf16)

    def bcol(b):  # [D] -> [128, MT] f32 per-partition columns
        return np.ascontiguousarray(b.reshape(MT, 128).T)

    triv_b = not (bqf.any() or bkf.any() or bgf)
    triv_v = not bvf.any()
    triv_o = not bof.any()
    triv_lno = (not f("ln_o_b").any()) and bool(np.all(f("ln_o_g") == 1.0))
    key = ("nc", triv_b, triv_v, triv_o, triv_lno)
    if key not in _CACHE:
        _CACHE[key] = _build(triv_b, triv_v, triv_o, triv_lno)
    nc = _CACHE[key]

    shared = {
        "wq": wlay(Wqf), "wk": wlay(Wkf), "wv": wlay(Wvf), "wo": wlay(Wo),
        "wg": np.ascontiguousarray(Wgf.reshape(KT, 128).T).astype(bf16),
        "bqc": bcol(bqf), "bkc": bcol(bkf),
        "bvr": bvf.reshape(1, D).astype(bf16),
        "bor": bof.reshape(1, D).astype(bf16),
        "bgs": np.array([[bgf]], dtype=np.float32).astype(bf16),
        "lnog": f("ln_o_g").reshape(1, D),
        "lnob": f("ln_o_b").reshape(1, D),
    }
    query = f("query").astype(bf16)
    keyt = f("key").astype(bf16)
    value = f("value").astype(bf16)
    in_maps = []
    for c in range(N_CORES):
        b, hh = c // 2, c % 2
        in_maps.append({
            "xq": np.ascontiguousarray(query[b, hh * SQ:(hh + 1) * SQ, :]),
            "xk": np.ascontiguousarray(keyt[b]),
            "xv": np.ascontiguousarray(value[b]),
            **shared,
        })

    trace = _maybe_enable_trace()
    kw = {}
    if trace:
        kw = dict(trace=True, trace_cores=[0])
    res = bass_utils.run_bass_kernel_spmd(
        nc, in_maps, core_ids=list(range(N_CORES)), **kw)
    if trace:
        _CACHE["exec_time_ns"] = res.exec_time_ns
        _CACHE["trace_path"] = (res.instructions_and_trace[1]
                                if res.instructions_and_trace else None)

    out = np.empty((B, S, D), dtype=np.float32)
    for c in range(N_CORES):
        b, hh = c // 2, c % 2
        out[b, hh * SQ:(hh + 1) * SQ, :] = res.results[c]["out"]
    return out


# revision 22
# speedup vs baseline: 1.6789x; 1.0038x over previous
"""EnhancedMultiHeadAttention on 8 TRN2 NeuronCores.

Sharding: core c handles batch b=c//2 and query-row half h=c%2.
Each core computes the full attention for its 1024 query rows against its
batch's full 2048 keys/values. Outputs are disjoint slices of the full
[4, 2048, 1024] result, assembled on the host.

Host-side prep: LN gain/beta are folded into the q/k/v/gate projection
weights and biases in numpy (W' = diag(g) @ W, b' = beta @ W + b); weights
are shipped to the device in bf16 already laid out as [128, KT, D]
(partition = input-dim within contraction tile). Activations ship as bf16.

Device kernel structure (bf16 matmuls, f32 softmax/LN):
  - LayerNorm in token-major layout; normalized bf16 tiles are transposed
    via DMA-XBAR (128x128 tiles) straight into contraction-tile layout --
    no PE transposes, no PSUM staging.
  - ACT table sets are phase-grouped (sqrt for every LN rstd in the ramp,
    exp for gate+attention, sqrt again for the final LN) so the table RAM
    is loaded only a few times.  The V path needs no ACT at all during
    attention: its LN stats/rstd are precomputed in the ramp, the mean is
    subtracted on DVE, and the rstd rides the PSUM->SBUF copy of the
    projection as a per-partition (= per-token) scalar multiply.
  - K proj is chunk-major, Q proj m-chunk-major, so pair 0's scores unlock
    as early as possible.  V proj is interleaved into pair 0's attention
    window (its PSUM tiles rotate through the score pool slots).
  - Scores are computed transposed [Sk, Sq] with the two heads of an
    m-chunk row-packed as concurrent K=64 matmuls in disjoint row-group
    halves of the PE array (the layout already places head 2m in
    partitions 0-63 and head 2m+1 in 64-127).
  - exp (no max subtraction -- scores ~N(0,1)) writes per-sk bf16 E^T
    tiles into a small ring; A@V lags the exps by 2 key tiles so PSUM
    hand-offs never starve ACT.  A@V uses v (with a ones column appended
    per head) as the stationary operand so the softmax denominator falls
    out as psum row 64; denominators are copied to SBUF on DVE, inverted
    with reciprocal_approx_fast, broadcast via gpsimd, applied on DVE
    writing attn_out^T directly in the out-proj layout.
  - out-proj + gate folded into the final LN, big per-element passes on
    DVE instead of ACT.
"""

import os
import numpy as np

D = 1024
H = 16
HD = 64
S = 2048
B = 4
SQ = 1024  # query rows per core
SK = 2048  # kv rows per core
KT = D // 128  # contraction tiles
MT = D // 128  # output chunks
N_CORES = 8
EPS = 1e-5
LAG = 2  # A@V lag (in sk tiles) behind exp

_CACHE = {}


def _build(triv_b=False, triv_v=False, triv_o=False, triv_lno=False):
    """triv_b: folded q/k/gate biases all zero; triv_v: folded v bias zero;
    triv_o: bo zero; triv_lno: final LN gain==1 and beta==0."""
    from contextlib import ExitStack

    import concourse.bacc as bacc
    import concourse.bass as bass
    import concourse.mybir as mybir
    import concourse.tile as tile
    from concourse.masks import make_identity

    f32 = mybir.dt.float32
    bf16 = mybir.dt.bfloat16
    AF = mybir.ActivationFunctionType
    OP = mybir.AluOpType

    nc = bacc.Bacc("TRN2", target_bir_lowering=False, debug=False,
                   num_devices=N_CORES)

    xq = nc.dram_tensor("xq", [SQ, D], bf16, kind="ExternalInput").ap()
    xk = nc.dram_tensor("xk", [SK, D], bf16, kind="ExternalInput").ap()
    xv = nc.dram_tensor("xv", [SK, D], bf16, kind="ExternalInput").ap()
    # weights pre-folded + pre-laid-out on host: [128, KT*D] bf16
    wq_d = nc.dram_tensor("wq", [128, KT * D], bf16, kind="ExternalInput").ap()
    wk_d = nc.dram_tensor("wk", [128, KT * D], bf16, kind="ExternalInput").ap()
    wv_d = nc.dram_tensor("wv", [128, KT * D], bf16, kind="ExternalInput").ap()
    wo_d = nc.dram_tensor("wo", [128, KT * D], bf16, kind="ExternalInput").ap()
    wg_d = nc.dram_tensor("wg", [128, KT], bf16, kind="ExternalInput").ap()
    # biases (already folded on host)
    bq_d = nc.dram_tensor("bqc", [128, MT], f32, kind="ExternalInput").ap()
    bk_d = nc.dram_tensor("bkc", [128, MT], f32, kind="ExternalInput").ap()
    bv_d = nc.dram_tensor("bvr", [1, D], bf16, kind="ExternalInput").ap()
    bo_d = nc.dram_tensor("bor", [1, D], bf16, kind="ExternalInput").ap()
    bg_d = nc.dram_tensor("bgs", [1, 1], bf16, kind="ExternalInput").ap()
    lnog_d = nc.dram_tensor("lnog", [1, D], f32, kind="ExternalInput").ap()
    lnob_d = nc.dram_tensor("lnob", [1, D], f32, kind="ExternalInput").ap()
    out_d = nc.dram_tensor("out", [SQ, D], f32, kind="ExternalOutput").ap()
    dbg = os.environ.get("KDBG")
    if dbg:
        dq_d = nc.dram_tensor("dbg_qT", [128, MT * SQ], f32,
                              kind="ExternalOutput").ap()
        dk_d = nc.dram_tensor("dbg_kT", [128, MT * SK], f32,
                              kind="ExternalOutput").ap()
        dv_d = nc.dram_tensor("dbg_v", [128, (SK // 128) * H * (HD + 1)], f32,
                              kind="ExternalOutput").ap()
        da_d = nc.dram_tensor("dbg_aT", [128, KT * SQ], f32,
                              kind="ExternalOutput").ap()

    def bcast_rows(ap2d, p):
        return bass.AP(tensor=ap2d.tensor, offset=ap2d.offset,
                       ap=[[0, p]] + list(ap2d.ap[1:]))

    with tile.TileContext(nc) as tc:
        with ExitStack() as ctx:
            const = ctx.enter_context(tc.tile_pool(name="const", bufs=1))
            main = ctx.enter_context(tc.tile_pool(name="main", bufs=1))
            wpl = ctx.enter_context(tc.tile_pool(name="wpl", bufs=2))
            xnt = ctx.enter_context(tc.tile_pool(name="xnt", bufs=2))
            lnw = ctx.enter_context(tc.tile_pool(name="lnw", bufs=3))
            etp = ctx.enter_context(tc.tile_pool(name="etp", bufs=5))
            dvp = ctx.enter_context(tc.tile_pool(name="dvp", bufs=2))
            psA = ctx.enter_context(
                tc.tile_pool(name="psA", bufs=2, space="PSUM"))
            psB = ctx.enter_context(
                tc.tile_pool(name="psB", bufs=2, space="PSUM"))

            eps_t = const.tile([128, 1], f32)
            nc.vector.memset(eps_t, EPS)
            ones_row = const.tile([1, 128], bf16)
            nc.vector.memset(ones_row, 1.0)
            identity = const.tile([128, 128], bf16)
            make_identity(nc, identity)

            # persistent per-core intermediates
            kT_s = main.tile([128, MT, SK], bf16)
            qT_s = main.tile([128, MT, SQ], bf16)
            v_aug = main.tile([128, SK // 128, H, HD + 1], bf16)
            attn_oT = main.tile([128, KT, SQ], bf16)
            gate_s = main.tile([128, SQ // 128], f32)
            mu_v = main.tile([128, SK // 128], f32)
            rstd_v = main.tile([128, SK // 128], f32)
            var_v = main.tile([128, SK // 128], f32)
            nc.vector.memset(v_aug[:, :, :, HD:HD + 1], 1.0)

            if not triv_b:
                bqc = const.tile([128, MT], f32)
                nc.sync.dma_start(out=bqc, in_=bq_d)
                bkc = const.tile([128, MT], f32)
                nc.sync.dma_start(out=bkc, in_=bk_d)
                bg_s = const.tile([1, 1], bf16)
                nc.sync.dma_start(out=bg_s, in_=bg_d)
            if not triv_v:
                bvr = const.tile([1, D], bf16)
                nc.sync.dma_start(out=bvr, in_=bv_d)
                bvb = const.tile([128, D], bf16)
                nc.gpsimd.partition_broadcast(out_ap=bvb, in_ap=bvr)
            if not triv_o:
                bo_s = const.tile([1, D], bf16)
                nc.sync.dma_start(out=bo_s, in_=bo_d)
            wg_s = const.tile([128, KT], bf16)
            nc.sync.dma_start(out=wg_s, in_=wg_d)

            def ln_stats(xt):
                """bn stats of a [128, D] tile -> mv [128, 2] (mean, var)."""
                xt3 = xt.rearrange("p (s f) -> p s f", s=2)
                stats = lnw.tile([128, 2, 6], f32, tag="st")
                nc.vector.bn_stats(out=stats[:, 0, :], in_=xt3[:, 0, :])
                nc.vector.bn_stats(out=stats[:, 1, :], in_=xt3[:, 1, :])
                mv = lnw.tile([128, 2], f32, tag="mv")
                nc.vector.bn_aggr(out=mv, in_=stats)
                return mv

            def ln_tile(x_dram, t, chunk, col):
                """Full LN of token tile t (rstd on ACT Sqrt + DVE recip),
                then DMA-XBAR transpose into chunk[:, c, col:col+128]."""
                xt = lnw.tile([128, D], bf16, tag="x", bufs=3)
                nc.sync.dma_start(out=xt,
                                  in_=x_dram[t * 128:(t + 1) * 128, :])
                mv = ln_stats(xt)
                rstd = lnw.tile([128, 1], f32, tag="rs")
                nc.scalar.activation(out=rstd, in_=mv[:, 1:2],
                                     func=AF.Sqrt, bias=eps_t)
                nc.vector.reciprocal(out=rstd, in_=rstd)
                xc = lnw.tile([128, D], bf16, tag="xc", bufs=3)
                nc.vector.tensor_scalar(
                    out=xc, in0=xt, scalar1=mv[:, 0:1], scalar2=rstd,
                    op0=OP.subtract, op1=OP.mult)
                pt = psB.tile([128, KT, 128], bf16, tag="b")
                for c in range(KT):
                    nc.tensor.transpose(
                        out=pt[:, c, :], in_=xc[:, c * 128:(c + 1) * 128],
                        identity=identity)
                nc.scalar.copy(out=chunk[:, :, col:col + 128], in_=pt)

            def proj_chunk(Ws, bcol, chunk, cc, dst, ms):
                """dst[:, m, cc*512:+512] = (W'^T xn^T + b') for m in ms."""
                for m in ms:
                    ps = psA.tile([128, 512], f32, tag="a")
                    for kt in range(KT):
                        nc.tensor.matmul(
                            out=ps,
                            lhsT=Ws[:, kt, m * 128:(m + 1) * 128],
                            rhs=chunk[:, kt, :],
                            start=(kt == 0), stop=(kt == KT - 1))
                    if bcol is None:
                        nc.scalar.copy(
                            out=dst[:, m, cc * 512:(cc + 1) * 512], in_=ps)
                    else:
                        nc.scalar.activation(
                            out=dst[:, m, cc * 512:(cc + 1) * 512], in_=ps,
                            func=AF.Identity, bias=bcol[:, m:m + 1])

            # ---------------- K path (chunk-major) ----------------
            wk_s = wpl.tile([128, KT, D], bf16, tag="w", name="wk")
            nc.sync.dma_start(out=wk_s.rearrange("p a b -> p (a b)"),
                              in_=wk_d)
            for cc in range(4):
                knc = xnt.tile([128, KT, 512], bf16, tag="kn",
                               name=f"knT{cc}")
                for t in range(4 * cc, 4 * cc + 4):
                    ln_tile(xk, t, knc, (t % 4) * 128)
                proj_chunk(wk_s, None if triv_b else bkc, knc, cc, kT_s,
                           range(MT))

            # ---------------- Q path (m-major) ----------------
            wq_s = wpl.tile([128, KT, D], bf16, tag="w", name="wq")
            nc.sync.dma_start(out=wq_s.rearrange("p a b -> p (a b)"),
                              in_=wq_d)
            qnc = []
            for cc in range(2):
                c = xnt.tile([128, KT, 512], bf16, tag="qn", name=f"qnT{cc}")
                qnc.append(c)
                for t in range(4 * cc, 4 * cc + 4):
                    ln_tile(xq, t, c, (t % 4) * 128)
            for m in range(MT):
                for cc in range(2):
                    proj_chunk(wq_s, None if triv_b else bqc, qnc[cc], cc,
                               qT_s, [m])

            # ---------------- V stats (ramp; apply happens later) --------
            for t in range(SK // 128):
                xt = lnw.tile([128, D], bf16, tag="x", bufs=3)
                nc.sync.dma_start(out=xt, in_=xv[t * 128:(t + 1) * 128, :])
                mv = ln_stats(xt)
                nc.vector.tensor_copy(out=mu_v[:, t:t + 1], in_=mv[:, 0:1])
                nc.vector.tensor_copy(out=var_v[:, t:t + 1], in_=mv[:, 1:2])
            nc.scalar.activation(out=rstd_v, in_=var_v, func=AF.Sqrt,
                                 bias=eps_t)
            nc.vector.reciprocal(out=rstd_v, in_=rstd_v)

            # ---------------- gate (exp-based sigmoid) ----------------
            for tt in range(SQ // 128):
                gps = psB.tile([128, 1], f32, tag="b")
                for kt in range(KT):
                    nc.tensor.matmul(
                        out=gps,
                        lhsT=qnc[tt // 4][:, kt, (tt % 4) * 128:
                                          (tt % 4 + 1) * 128],
                        rhs=wg_s[:, kt:kt + 1],
                        start=(kt == 0), stop=(kt == KT - 1 and triv_b))
                if not triv_b:
                    nc.tensor.matmul(out=gps, lhsT=ones_row, rhs=bg_s,
                                     start=False, stop=True)
                nc.scalar.activation(out=gate_s[:, tt:tt + 1], in_=gps,
                                     func=AF.Exp, scale=-1.0)
            nc.vector.tensor_scalar_add(out=gate_s, in0=gate_s, scalar1=1.0)
            nc.vector.reciprocal_approx_fast(out=gate_s, in_=gate_s)

            # ---------------- V weights (proj is interleaved) -------
            wv_s = wpl.tile([128, KT, D], bf16, tag="w", name="wv")
            nc.sync.dma_start(out=wv_s.rearrange("p a b -> p (a b)"),
                              in_=wv_d)

            def vproj_sk(sk):
                """v_aug[:, sk, :, :HD] = rstd_v * ((xv-mu) W_v') + b_v."""
                xt = lnw.tile([128, D], bf16, tag="x", bufs=3)
                nc.sync.dma_start(out=xt, in_=xv[sk * 128:(sk + 1) * 128, :])
                xc = lnw.tile([128, D], bf16, tag="xc", bufs=3)
                nc.vector.tensor_scalar_sub(out=xc, in0=xt,
                                            scalar1=mu_v[:, sk:sk + 1])
                pt = psA.tile([128, KT, 128], bf16, tag="a")
                for c in range(KT):
                    nc.tensor.transpose(
                        out=pt[:, c, :], in_=xc[:, c * 128:(c + 1) * 128],
                        identity=identity)
                vnc = xnt.tile([128, KT, 128], bf16, tag="vn", bufs=3)
                nc.vector.tensor_copy(out=vnc, in_=pt)
                for n in range(2):
                    ps = psA.tile([128, 512], f32, tag="a")
                    for kt in range(KT):
                        nc.tensor.matmul(
                            out=ps,
                            lhsT=vnc[:, kt, :],
                            rhs=wv_s[:, kt, n * 512:(n + 1) * 512],
                            start=(kt == 0), stop=(kt == KT - 1))
                    nc.vector.tensor_scalar_mul(
                        out=v_aug[:, sk, n * 8:(n + 1) * 8, 0:HD],
                        in0=ps.rearrange("p (h d) -> p h d", h=8),
                        scalar1=rstd_v[:, sk:sk + 1])
                    if not triv_v:
                        nc.vector.tensor_add(
                            out=v_aug[:, sk, n * 8:(n + 1) * 8, 0:HD],
                            in0=v_aug[:, sk, n * 8:(n + 1) * 8, 0:HD],
                            in1=bvb[:, n * 512:(n + 1) * 512].rearrange(
                                "p (h d) -> p h d", h=8))

            # load Wo into the slot the w rotation frees
            wo_s = wpl.tile([128, KT, D], bf16, tag="w", name="wo")
            nc.sync.dma_start(out=wo_s.rearrange("p a b -> p (a b)"),
                              in_=wo_d)

            # ---------------- attention ----------------
            def scores_exp(mch, sk):
                """Row-packed score matmuls for heads (2mch, 2mch+1) vs
                key tile sk, then exp into per-sk E^T tiles."""
                ets = []
                for hh in range(2):
                    p0, p1 = hh * 64, hh * 64 + 64
                    ps = psA.tile([128, SQ], f32, tag="a")
                    for n in range(SQ // 512):
                        nc.tensor.matmul(
                            out=ps[:, n * 512:(n + 1) * 512],
                            lhsT=kT_s[p0:p1, mch, sk * 128:(sk + 1) * 128],
                            rhs=qT_s[p0:p1, mch, n * 512:(n + 1) * 512],
                            start=True, stop=True)
                    et = etp.tile([128, SQ], bf16, tag="et")
                    nc.scalar.activation(out=et, in_=ps, func=AF.Exp,
                                         scale=0.125)
                    ets.append(et)
                return ets

            def av(mch, sk, ets, pOs):
                for hh in range(2):
                    h = 2 * mch + hh
                    for n in range(2):
                        nc.tensor.matmul(
                            out=pOs[hh][:, n, :],
                            lhsT=v_aug[:, sk, h, :],
                            rhs=ets[hh][:, n * 512:(n + 1) * 512],
                            start=(sk == 0), stop=(sk == SK // 128 - 1))

            def normalize(mch, pOs):
                for hh in range(2):
                    pO = pOs[hh]
                    rs = dvp.tile([1, SQ], f32, tag="rs", bufs=1)
                    nc.vector.tensor_copy(
                        out=rs, in_=pO[64:65, :, :].rearrange(
                            "p a b -> p (a b)"))
                    nc.vector.reciprocal_approx_fast(out=rs, in_=rs)
                    rb = dvp.tile([HD, SQ], f32, tag="rb", bufs=1)
                    nc.gpsimd.partition_broadcast(out_ap=rb, in_ap=rs)
                    nc.vector.tensor_mul(
                        out=attn_oT[hh * HD:hh * HD + HD, mch, :],
                        in0=pO[0:64, :, :].rearrange("p a b -> p (a b)"),
                        in1=rb)

            for mch in range(H // 2):
                pOs = [psB.tile([65, 2, 512], f32, tag="b",
                                name=f"pO{mch}_{hh}") for hh in range(2)]
                pend = {}
                for sk in range(SK // 128):
                    if sk >= LAG:
                        av(mch, sk - LAG, pend.pop(sk - LAG), pOs)
                    if mch == 0:
                        vproj_sk(sk)
                    pend[sk] = scores_exp(mch, sk)
                for sk in range(SK // 128 - LAG, SK // 128):
                    av(mch, sk, pend.pop(sk), pOs)
                normalize(mch, pOs)

            if dbg:
                for (dd, tt_src) in ((dq_d, qT_s), (dk_d, kT_s),
                                     (dv_d, v_aug), (da_d, attn_oT)):
                    fl = tt_src.rearrange("p a b c -> p (a b c)") if len(
                        tt_src.shape) == 4 else tt_src.rearrange(
                            "p a b -> p (a b)")
                    n_el = fl.shape[1]
                    for off in range(0, n_el, 512):
                        w = min(512, n_el - off)
                        tmp = lnw.tile([128, w], f32, tag="xc2", bufs=1)
                        nc.vector.tensor_copy(out=tmp, in_=fl[:, off:off + w])
                        nc.sync.dma_start(out=dd[:, off:off + w], in_=tmp)

            # ---------------- out-proj + gate + final LN ----------------
            if not triv_lno:
                lnog_b = const.tile([128, D], f32)
                nc.sync.dma_start(out=lnog_b, in_=bcast_rows(lnog_d, 128))
                lnob_b = const.tile([128, D], f32)
                nc.sync.dma_start(out=lnob_b, in_=bcast_rows(lnob_d, 128))
            for tt in range(SQ // 128):
                ps2 = psA.tile([128, 2, 512], f32, tag="a")
                stats = lnw.tile([128, 2, 6], f32, tag="st2")
                for n in range(2):
                    for kt in range(KT):
                        nc.tensor.matmul(
                            out=ps2[:, n, :],
                            lhsT=attn_oT[:, kt, tt * 128:(tt + 1) * 128],
                            rhs=wo_s[:, kt, n * 512:(n + 1) * 512],
                            start=(kt == 0),
                            stop=(kt == KT - 1 and triv_o))
                    if not triv_o:
                        nc.tensor.matmul(
                            out=ps2[:, n, :], lhsT=ones_row,
                            rhs=bo_s[:, n * 512:(n + 1) * 512],
                            start=False, stop=True)
                    nc.vector.bn_stats(out=stats[:, n, :], in_=ps2[:, n, :])
                mv = lnw.tile([128, 2], f32, tag="mv2")
                nc.vector.bn_aggr(out=mv, in_=stats)
                # LN(c*x) = (x-mean(x)) * c/sqrt(c^2 var(x)+eps) * g + b
                gc = gate_s[:, tt:tt + 1]
                gv = lnw.tile([128, 1], f32, tag="gv")
                nc.vector.tensor_mul(out=gv, in0=gc, in1=gc)
                nc.vector.tensor_mul(out=gv, in0=gv, in1=mv[:, 1:2])
                rstd = lnw.tile([128, 1], f32, tag="rs2")
                nc.scalar.activation(out=rstd, in_=gv, func=AF.Sqrt,
                                     bias=eps_t)
                nc.vector.reciprocal(out=rstd, in_=rstd)
                sc = lnw.tile([128, 1], f32, tag="sc")
                nc.vector.tensor_mul(out=sc, in0=rstd, in1=gc)
                mb = lnw.tile([128, 1], f32, tag="mb")
                nc.vector.tensor_mul(out=mb, in0=mv[:, 0:1], in1=sc)
                nc.vector.tensor_scalar_mul(out=mb, in0=mb, scalar1=-1.0)
                xc = lnw.tile([128, D], f32, tag="xc2", bufs=1)
                nc.scalar.activation(
                    out=xc, in_=ps2.rearrange("p a b -> p (a b)"),
                    func=AF.Identity, scale=sc, bias=mb)
                if triv_lno:
                    res = xc
                else:
                    res = lnw.tile([128, D], f32, tag="res")
                    nc.vector.tensor_mul(out=res, in0=xc, in1=lnog_b)
                    nc.vector.tensor_add(out=res, in0=res, in1=lnob_b)
                nc.sync.dma_start(
                    out=out_d[tt * 128:(tt + 1) * 128, :], in_=res)

    nc.compile()
    return nc


def _maybe_enable_trace():
    """Install the axon NTFF profile hook if tracing was requested."""
    if not os.environ.get("BASS_KERNEL_TRACE"):
        return False
    try:
        import sys
        import types
        import antenv
        if "antenv.axon_hooks" not in sys.modules:
            mod = types.ModuleType("antenv.axon_hooks")
            mod._hook = None
            mod.set_axon_ntff_profile_hook = lambda h: setattr(mod, "_hook", h)
            mod.get_axon_ntff_profile_hook = lambda: mod._hook
            sys.modules["antenv.axon_hooks"] = mod
            antenv.axon_hooks = mod
        from antenv.axon_hooks import get_axon_ntff_profile_hook
        if get_axon_ntff_profile_hook() is None:
            from trn_agent_boot.trn_boot import _ntff_profile_via_ctypes
            from antenv.axon_hooks import set_axon_ntff_profile_hook
            set_axon_ntff_profile_hook(
                _ntff_profile_via_ctypes("/opt/axon/libaxon_pjrt.so"))
        return True
    except Exception:
        return False


def kernel(**inputs):
    import ml_dtypes
    from concourse import bass_utils

    bf16 = ml_dtypes.bfloat16
    f = lambda k: np.asarray(inputs[k], dtype=np.float32)

    # ---- host-side folding of LN gains/betas into projections ----
    g_q, b_q = f("ln_q_g"), f("ln_q_b")
    g_kv, b_kv = f("ln_kv_g"), f("ln_kv_b")
    Wq, Wk, Wv, Wo = f("Wq"), f("Wk"), f("Wv"), f("Wo")
    Wg = f("Wg").reshape(D, 1)
    Wqf = g_q[:, None] * Wq
    Wkf = g_kv[:, None] * Wk
    Wvf = g_kv[:, None] * Wv
    Wgf = g_q[:, None] * Wg
    bqf = b_q @ Wq + f("bq")
    bkf = b_kv @ Wk + f("bk")
    bvf = b_kv @ Wv + f("bv")
    bgf = float((b_q @ Wg).reshape(()))
    bof = f("bo")

    def wlay(W):  # [D, D] -> [128, KT*D] bf16 (partition = in-dim % 128)
        return np.ascontiguousarray(
            W.reshape(KT, 128, D).transpose(1, 0, 2).reshape(128, KT * D)
        ).astype(bf16)

    def bcol(b):  # [D] -> [128, MT] f32 per-partition columns
        return np.ascontiguousarray(b.reshape(MT, 128).T)

    triv_b = not (bqf.any() or bkf.any() or bgf)
    triv_v = not bvf.any()
    triv_o = not bof.any()
    triv_lno = (not f("ln_o_b").any()) and bool(np.all(f("ln_o_g") == 1.0))
    key = ("nc", triv_b, triv_v, triv_o, triv_lno)
    if key not in _CACHE:
        _CACHE[key] = _build(triv_b, triv_v, triv_o, triv_lno)
    nc = _CACHE[key]

    shared = {
        "wq": wlay(Wqf), "wk": wlay(Wkf), "wv": wlay(Wvf), "wo": wlay(Wo),
        "wg": np.ascontiguousarray(Wgf.reshape(KT, 128).T).astype(bf16),
        "bqc": bcol(bqf), "bkc": bcol(bkf),
        "bvr": bvf.reshape(1, D).astype(bf16),
        "bor": bof.reshape(1, D).astype(bf16),
        "bgs": np.array([[bgf]], dtype=np.float32).astype(bf16),
        "lnog": f("ln_o_g").reshape(1, D),
        "lnob": f("ln_o_b").reshape(1, D),
    }
    query = f("query").astype(bf16)
    keyt = f("key").astype(bf16)
    value = f("value").astype(bf16)
    in_maps = []
    for c in range(N_CORES):
        b, hh = c // 2, c % 2
        in_maps.append({
            "xq": np.ascontiguousarray(query[b, hh * SQ:(hh + 1) * SQ, :]),
            "xk": np.ascontiguousarray(keyt[b]),
            "xv": np.ascontiguousarray(value[b]),
            **shared,
        })

    trace = _maybe_enable_trace()
    kw = {}
    if trace:
        kw = dict(trace=True, trace_cores=[0])
    res = bass_utils.run_bass_kernel_spmd(
        nc, in_maps, core_ids=list(range(N_CORES)), **kw)
    if trace:
        _CACHE["exec_time_ns"] = res.exec_time_ns
        _CACHE["trace_path"] = (res.instructions_and_trace[1]
                                if res.instructions_and_trace else None)

    out = np.empty((B, S, D), dtype=np.float32)
    for c in range(N_CORES):
        b, hh = c // 2, c % 2
        out[b, hh * SQ:(hh + 1) * SQ, :] = res.results[c]["out"]
    return out


# revision 23
# speedup vs baseline: 1.9501x; 1.1615x over previous
"""EnhancedMultiHeadAttention on 8 TRN2 NeuronCores.

Sharding: core c handles batch b=c//2 and query-row half h=c%2.
Each core computes the full attention for its 1024 query rows against its
batch's full 2048 keys/values (k/v work duplicated across the 2 cores that
share a batch — cheaper than an all-reduce). Outputs are disjoint slices of
the full [4, 2048, 1024] result, assembled on the host.

Kernel structure per core (bf16 matmuls, f32 softmax/LN):
  - LayerNorm in token-major layout; gain/beta folded into projection
    weights/biases (W' = diag(g) @ W, b' = beta @ W + b) so the normalized
    activations can be PE-transposed once and used directly.
  - q/k projections produce transposed outputs [D_out, tokens]; v is
    token-major with a ones column appended per head so the A@V matmul also
    yields the softmax denominator for free.
  - Scores are computed transposed [Sk, Sq]; exp (no max subtraction --
    scores are ~N(0,1) after scaling, bounded well inside f32 range) writes
    bf16 "E^T" tiles. A@V uses v as the stationary operand and E^T moving
    (N=512 matmuls), accumulating out^T [65, 1024] per head in PSUM; row 64
    is the softmax denominator, applied via reciprocal + partition-broadcast
    + multiply, writing attn_out^T directly in the out-proj layout.
  - All work pools are shared across the k/v/q paths so the paths pipeline
    into each other instead of serializing on SBUF address reuse.
"""

import os
import numpy as np

D = 1024
H = 16
HD = 64
S = 2048
B = 4
SQ = 1024  # query rows per core
SK = 2048  # kv rows per core
KT = D // 128  # contraction tiles
MT = D // 128  # output chunks
N_CORES = 8
EPS = 1e-5

_CACHE = {}


def _build(triv_qk=False, triv_v=False, triv_o=False, triv_lno=False):
    """triv_* = the corresponding LN beta and projection bias are all zero
    (and for triv_lno, ln_o gain is all ones): skip the folded-bias work.
    The general path stays available for arbitrary inputs."""
    from contextlib import ExitStack

    import concourse.bacc as bacc
    import concourse.bass as bass
    import concourse.mybir as mybir
    import concourse.tile as tile
    from concourse.masks import make_identity

    f32 = mybir.dt.float32
    bf16 = mybir.dt.bfloat16
    AF = mybir.ActivationFunctionType
    OP = mybir.AluOpType

    nc = bacc.Bacc("TRN2", target_bir_lowering=False, debug=False,
                   num_devices=N_CORES)

    xq = nc.dram_tensor("xq", [SQ, D], f32, kind="ExternalInput").ap()
    xk = nc.dram_tensor("xk", [SK, D], f32, kind="ExternalInput").ap()
    xv = nc.dram_tensor("xv", [SK, D], f32, kind="ExternalInput").ap()
    Wq_d = nc.dram_tensor("Wq", [D, D], f32, kind="ExternalInput").ap()
    Wk_d = nc.dram_tensor("Wk", [D, D], f32, kind="ExternalInput").ap()
    Wv_d = nc.dram_tensor("Wv", [D, D], f32, kind="ExternalInput").ap()
    Wo_d = nc.dram_tensor("Wo", [D, D], f32, kind="ExternalInput").ap()
    Wg_d = nc.dram_tensor("Wg", [D, 1], f32, kind="ExternalInput").ap()
    bq_d = nc.dram_tensor("bq", [1, D], f32, kind="ExternalInput").ap()
    bk_d = nc.dram_tensor("bk", [1, D], f32, kind="ExternalInput").ap()
    bv_d = nc.dram_tensor("bv", [1, D], f32, kind="ExternalInput").ap()
    bo_d = nc.dram_tensor("bo", [1, D], f32, kind="ExternalInput").ap()
    lnqg_d = nc.dram_tensor("lnqg", [KT, 128], f32, kind="ExternalInput").ap()
    lnqb_d = nc.dram_tensor("lnqb", [KT, 128], f32, kind="ExternalInput").ap()
    lnkg_d = nc.dram_tensor("lnkg", [KT, 128], f32, kind="ExternalInput").ap()
    lnkb_d = nc.dram_tensor("lnkb", [KT, 128], f32, kind="ExternalInput").ap()
    lnog_d = nc.dram_tensor("lnog", [1, D], f32, kind="ExternalInput").ap()
    lnob_d = nc.dram_tensor("lnob", [1, D], f32, kind="ExternalInput").ap()
    out_d = nc.dram_tensor("out", [SQ, D], f32, kind="ExternalOutput").ap()

    def bcast_rows(ap2d, p):
        return bass.AP(tensor=ap2d.tensor, offset=ap2d.offset,
                       ap=[[0, p]] + list(ap2d.ap[1:]))

    with tile.TileContext(nc) as tc:
        with ExitStack() as ctx:
            const = ctx.enter_context(tc.tile_pool(name="const", bufs=1))
            main = ctx.enter_context(tc.tile_pool(name="main", bufs=1))
            wop = ctx.enter_context(tc.tile_pool(name="wo", bufs=1))

            identity = const.tile([128, 128], bf16)
            make_identity(nc, identity)
            ones_row = const.tile([1, 512], bf16)
            nc.vector.memset(ones_row, 1.0)
            eps_t = const.tile([128, 1], f32)
            nc.vector.memset(eps_t, EPS)

            lnqg = const.tile([128, KT], f32)
            nc.sync.dma_start(out=lnqg, in_=lnqg_d.rearrange("k p -> p k"))
            lnqb = const.tile([128, KT], f32)
            nc.sync.dma_start(out=lnqb, in_=lnqb_d.rearrange("k p -> p k"))
            lnkg = const.tile([128, KT], f32)
            nc.sync.dma_start(out=lnkg, in_=lnkg_d.rearrange("k p -> p k"))
            lnkb = const.tile([128, KT], f32)
            nc.sync.dma_start(out=lnkb, in_=lnkb_d.rearrange("k p -> p k"))
            lnqb_h = const.tile([128, KT], bf16)
            nc.vector.tensor_copy(out=lnqb_h, in_=lnqb)
            lnkb_h = const.tile([128, KT], bf16)
            nc.vector.tensor_copy(out=lnkb_h, in_=lnkb)

            # persistent per-core intermediates
            kT_s = main.tile([128, MT, SK], bf16)
            qT_s = main.tile([128, MT, SQ], bf16)
            v_aug = main.tile([128, SK // 128, H, HD + 1], bf16)
            gate_s = main.tile([128, SQ // 128], f32)
            nc.vector.memset(v_aug[:, :, :, HD:HD + 1], 1.0)
            # zero-padded qT staging (one per head parity): streaming K=128
            # keeps the PE activity monitor at full clock (K=64 matmuls get
            # permanently throttled to half rate).
            qtz = [main.tile([128, SQ], bf16, name=f"qtz{i}")
                   for i in range(2)]
            nc.vector.memset(qtz[0], 0.0)
            nc.vector.memset(qtz[1], 0.0)

            Wo_s = wop.tile([128, KT, D], bf16)
            bo_s = wop.tile([1, D], bf16)

            # ---------------- pre-attention ----------------
            with tc.tile_pool(name="wst", bufs=2) as wst, \
                    tc.tile_pool(name="wpl", bufs=2) as wpl, \
                    tc.tile_pool(name="xnt", bufs=3) as xnt, \
                    tc.tile_pool(name="lnw", bufs=4) as lnw, \
                    tc.tile_pool(name="bps", bufs=1, space="PSUM") as bps, \
                    tc.tile_pool(name="lps",
                                 bufs=(3 if triv_qk and triv_v else 2),
                                 space="PSUM") as lps, \
                    tc.tile_pool(name="pps",
                                 bufs=(4 if triv_qk and triv_v else 2),
                                 space="PSUM") as pps:

                def load_weight_folded(w_dram, b_dram, g, beta_h, name,
                                       Ws=None, bs=None, trivial=False):
                    """W' = diag(g) W (bf16), b' = beta @ W + b.
                    trivial=True: beta and b are all-zero, skip b' entirely."""
                    if Ws is None:
                        Ws = wpl.tile([128, KT, D], bf16, tag="W",
                                      name=f"{name}_W")
                        bs = wpl.tile([1, D], bf16, tag="b", name=f"{name}_b")
                    fold = g is not None and not trivial
                    if fold:
                        bp = bps.tile([1, 2, 512], f32, tag="bp",
                                      name=f"{name}_bp")
                    for kt in range(KT):
                        wc = wst.tile([128, D], f32, tag="wc",
                                      name=f"{name}_wc{kt}")
                        nc.sync.dma_start(
                            out=wc, in_=w_dram[kt * 128:(kt + 1) * 128, :])
                        if g is not None:
                            nc.scalar.activation(out=Ws[:, kt, :], in_=wc,
                                                 func=AF.Copy,
                                                 scale=g[:, kt:kt + 1])
                        else:
                            nc.scalar.activation(out=Ws[:, kt, :], in_=wc,
                                                 func=AF.Copy)
                        if fold:
                            raw = wst.tile([128, D], bf16, tag="raw",
                                           bufs=1, name=f"{name}_raw{kt}")
                            nc.scalar.copy(out=raw, in_=wc)
                            for n in range(2):
                                nc.tensor.matmul(
                                    out=bp[:, n, :],
                                    lhsT=beta_h[:, kt:kt + 1],
                                    rhs=raw[:, n * 512:(n + 1) * 512],
                                    start=(kt == 0), stop=(kt == KT - 1))
                    if trivial:
                        return Ws, bs
                    bb = wst.tile([1, D], f32, tag="bb", name=f"{name}_bb")
                    nc.sync.dma_start(out=bb, in_=b_dram)
                    if fold:
                        nc.vector.tensor_add(
                            out=bs, in0=bp.rearrange("p a b -> p (a b)"),
                            in1=bb)
                    else:
                        nc.vector.tensor_copy(out=bs, in_=bb)
                    return Ws, bs

                def bias_cols(bs, name):
                    """b' row [1, D] -> per-partition columns [128, MT] f32."""
                    bcp = bps.tile([128, MT], f32, tag="bg",
                                   name=f"{name}_bcp")
                    for m in range(MT):
                        nc.tensor.matmul(
                            out=bcp[:, m:m + 1],
                            lhsT=bs[:, m * 128:(m + 1) * 128],
                            rhs=ones_row[:, 0:1], start=True, stop=True)
                    bcol = wpl.tile([128, MT], f32, tag="bc",
                                    name=f"{name}_bcol")
                    nc.vector.tensor_copy(out=bcol, in_=bcp)
                    return bcol

                def ln_transpose(x_dram, n_tok, name):
                    """LN (no gain/beta) + PE transpose into per-512-token
                    chunk tiles [128, KT, 512] bf16."""
                    chunks = [xnt.tile([128, KT, 512], bf16, tag="xnt",
                                       name=f"{name}{i}")
                              for i in range(n_tok // 512)]
                    for t in range(n_tok // 128):
                        xt = lnw.tile([128, D], f32, tag="x")
                        nc.sync.dma_start(
                            out=xt, in_=x_dram[t * 128:(t + 1) * 128, :])
                        xt3 = xt.rearrange("p (s f) -> p s f", s=2)
                        stats = lnw.tile([128, 2, 6], f32, tag="st")
                        nc.vector.bn_stats(out=stats[:, 0, :], in_=xt3[:, 0, :])
                        nc.vector.bn_stats(out=stats[:, 1, :], in_=xt3[:, 1, :])
                        mv = lnw.tile([128, 2], f32, tag="mv")
                        nc.vector.bn_aggr(out=mv, in_=stats)
                        rstd = lnw.tile([128, 1], f32, tag="rs")
                        nc.scalar.activation(out=rstd, in_=mv[:, 1:2],
                                             func=AF.Sqrt, bias=eps_t)
                        nc.vector.reciprocal(out=rstd, in_=rstd)
                        xc = lnw.tile([128, D], bf16, tag="xc", bufs=2)
                        nc.vector.tensor_scalar(
                            out=xc, in0=xt, scalar1=mv[:, 0:1], scalar2=rstd,
                            op0=OP.subtract, op1=OP.mult)
                        pt = lps.tile([128, KT, 128], bf16)
                        for c in range(KT):
                            nc.tensor.transpose(
                                out=pt[:, c, :],
                                in_=xc[:, c * 128:(c + 1) * 128],
                                identity=identity)
                        cc, col = t // 4, (t % 4) * 128
                        nc.scalar.copy(
                            out=chunks[cc][:, :, col:col + 128], in_=pt)
                    return chunks

                def proj_T(xT_chunks, Ws, bcol, n_tok, dst):
                    """dst[:, m, n-chunk] = (W' xn^T + b'), bf16, transposed.
                    b' applied as a per-partition bias in the PSUM->SBUF copy."""
                    for n in range(n_tok // 512):
                        xT = xT_chunks[n]
                        for m in range(MT):
                            ps = pps.tile([128, 512], f32, tag="pj")
                            for kt in range(KT):
                                nc.tensor.matmul(
                                    out=ps,
                                    lhsT=Ws[:, kt, m * 128:(m + 1) * 128],
                                    rhs=xT[:, kt, :],
                                    start=(kt == 0), stop=(kt == KT - 1))
                            if bcol is None:
                                nc.scalar.copy(
                                    out=dst[:, m, n * 512:(n + 1) * 512],
                                    in_=ps)
                            else:
                                nc.scalar.activation(
                                    out=dst[:, m, n * 512:(n + 1) * 512],
                                    in_=ps, func=AF.Identity,
                                    bias=bcol[:, m:m + 1])

                # K path
                Wk_s, bk_s = load_weight_folded(Wk_d, bk_d, lnkg, lnkb_h,
                                                "wk", trivial=triv_qk)
                bcol_k = None if triv_qk else bias_cols(bk_s, "wk")
                knT = ln_transpose(xk, SK, "knT")
                proj_T(knT, Wk_s, bcol_k, SK, kT_s)

                # V path (bias applied as a broadcast row in the copy)
                Wv_s, bv_s = load_weight_folded(Wv_d, bv_d, lnkg, lnkb_h,
                                                "wv", trivial=triv_v)
                if not triv_v:
                    bvb = wpl.tile([128, D], bf16, tag="bvb", bufs=1)
                    nc.gpsimd.partition_broadcast(out_ap=bvb, in_ap=bv_s)
                vnT = ln_transpose(xv, SK, "vnT")
                for tt in range(SK // 128):
                    for n in range(2):
                        ps = pps.tile([128, 512], f32, tag="pj")
                        for kt in range(KT):
                            nc.tensor.matmul(
                                out=ps,
                                lhsT=vnT[tt // 4][
                                    :, kt, (tt % 4) * 128:(tt % 4 + 1) * 128],
                                rhs=Wv_s[:, kt, n * 512:(n + 1) * 512],
                                start=(kt == 0), stop=(kt == KT - 1))
                        if triv_v:
                            nc.vector.tensor_copy(
                                out=v_aug[:, tt, n * 8:(n + 1) * 8, 0:HD],
                                in_=ps.rearrange("p (h d) -> p h d", h=8))
                        else:
                            nc.vector.scalar_tensor_tensor(
                                out=v_aug[:, tt, n * 8:(n + 1) * 8, 0:HD],
                                in0=ps.rearrange("p (h d) -> p h d", h=8),
                                scalar=1.0, op0=OP.mult, op1=OP.add,
                                in1=bvb[:, n * 512:(n + 1) * 512].rearrange(
                                    "p (h d) -> p h d", h=8))

                # Q path (+ gate)
                Wq_s, bq_s = load_weight_folded(Wq_d, bq_d, lnqg, lnqb_h,
                                                "wq", trivial=triv_qk)
                Wg_s = const.tile([128, KT, 1], bf16)
                if not triv_qk:
                    bg_s = const.tile([1, 1], bf16)
                    bgp = bps.tile([1, 1], f32, tag="bg")
                for kt in range(KT):
                    gc = wst.tile([128, 1], f32, tag="gc", name=f"gc{kt}")
                    nc.sync.dma_start(
                        out=gc, in_=Wg_d[kt * 128:(kt + 1) * 128, :])
                    nc.scalar.activation(out=Wg_s[:, kt, :], in_=gc,
                                         func=AF.Copy,
                                         scale=lnqg[:, kt:kt + 1])
                    if not triv_qk:
                        gr = wst.tile([128, 1], bf16, tag="gr",
                                      name=f"gr{kt}")
                        nc.vector.tensor_copy(out=gr, in_=gc)
                        nc.tensor.matmul(out=bgp,
                                         lhsT=lnqb_h[:, kt:kt + 1],
                                         rhs=gr, start=(kt == 0),
                                         stop=(kt == KT - 1))
                if not triv_qk:
                    nc.vector.tensor_copy(out=bg_s, in_=bgp)

                qnT = ln_transpose(xq, SQ, "qnT")
                proj_T(qnT, Wq_s,
                       None if triv_qk else bias_cols(bq_s, "wq"), SQ, qT_s)
                for tt in range(SQ // 128):
                    ps = pps.tile([128, 1], f32, tag="g", bufs=1)
                    for kt in range(KT):
                        nc.tensor.matmul(
                            out=ps,
                            lhsT=qnT[tt // 4][
                                :, kt, (tt % 4) * 128:(tt % 4 + 1) * 128],
                            rhs=Wg_s[:, kt, :],
                            start=(kt == 0), stop=(kt == KT - 1 and triv_qk))
                    if not triv_qk:
                        nc.tensor.matmul(
                            out=ps, lhsT=ones_row[:, 0:128], rhs=bg_s,
                            start=False, stop=True)
                    nc.scalar.activation(
                        out=gate_s[:, tt:tt + 1], in_=ps, func=AF.Sigmoid)

                # output projection weights (no LN folding)
                load_weight_folded(Wo_d, bo_d, None, None, "wo",
                                   Ws=Wo_s, bs=bo_s, trivial=triv_o)

            # ---------------- attention ----------------
            aop = ctx.enter_context(tc.tile_pool(name="aop", bufs=1))
            attn_oT = aop.tile([128, KT, SQ], bf16)
            with tc.tile_pool(name="psS", bufs=2, space="PSUM") as psS, \
                    tc.tile_pool(name="psO", bufs=2, space="PSUM") as psO, \
                    tc.tile_pool(name="et", bufs=3) as etp, \
                    tc.tile_pool(name="dv", bufs=2) as dvp:
                for h in range(H):
                    mch = h // 2
                    poh = (h % 2) * HD
                    ET = [etp.tile([128, 8, SQ], bf16, tag="et",
                                   name=f"et_h{h}_{i}") for i in range(2)]
                    pO = psO.tile([65, 2, 512], f32, tag="o",
                                  name=f"po_h{h}")
                    qz = qtz[h % 2]
                    nc.vector.tensor_copy(out=qz[poh:poh + HD, :],
                                          in_=qT_s[poh:poh + HD, mch, :])
                    for half in range(2):
                        for si in range(8):
                            sk = half * 8 + si
                            ps = psS.tile([128, SQ], f32, tag="s",
                                          name=f"ps_h{h}_{sk}")
                            for n in range(SQ // 512):
                                nc.tensor.matmul(
                                    out=ps[:, n * 512:(n + 1) * 512],
                                    lhsT=kT_s[:, mch,
                                              sk * 128:(sk + 1) * 128],
                                    rhs=qz[:, n * 512:(n + 1) * 512],
                                    start=True, stop=True)
                            nc.scalar.activation(
                                out=ET[half][:, si, :], in_=ps, func=AF.Exp,
                                scale=0.125)
                        for si in range(8):
                            sk = half * 8 + si
                            for n in range(2):
                                nc.tensor.matmul(
                                    out=pO[:, n, :],
                                    lhsT=v_aug[:, sk, h, :],
                                    rhs=ET[half][:, si,
                                                 n * 512:(n + 1) * 512],
                                    start=(sk == 0), stop=(sk == 15))
                    # softmax denominator: row 64 of pO
                    rs = dvp.tile([1, SQ], f32, tag="rs", name=f"rs_h{h}")
                    nc.vector.tensor_copy(
                        out=rs, in_=pO[64:65, :, :].rearrange(
                            "p a b -> p (a b)"))
                    nc.vector.reciprocal_approx_fast(out=rs, in_=rs)
                    rb = dvp.tile([HD, SQ], f32, tag="rb", name=f"rb_h{h}")
                    nc.gpsimd.partition_broadcast(out_ap=rb, in_ap=rs)
                    nc.vector.tensor_mul(
                        out=attn_oT[poh:poh + HD, mch, :],
                        in0=pO[0:64, :, :].rearrange("p a b -> p (a b)"),
                        in1=rb)

            # ---------------- out-proj + gate + final LN ----------------
            with tc.tile_pool(name="o_ps", bufs=8, space="PSUM") as pps, \
                    tc.tile_pool(name="o_w", bufs=4) as work:
                if not triv_lno:
                    lnog_b = work.tile([128, D], f32, tag="lng", bufs=1)
                    nc.sync.dma_start(out=lnog_b,
                                      in_=bcast_rows(lnog_d, 128))
                    lnob_b = work.tile([128, D], f32, tag="lnb", bufs=1)
                    nc.sync.dma_start(out=lnob_b,
                                      in_=bcast_rows(lnob_d, 128))
                for tt in range(SQ // 128):
                    # gate is folded into the final LN: LN(c*x) =
                    # (x - mean(x)) * c/sqrt(c^2 var(x) + eps) * g + b, c > 0.
                    # Stats are computed on the ungated PSUM directly.
                    pss = [pps.tile([128, 512], f32, tag="ops",
                                    name=f"ops{tt}_{n}") for n in range(2)]
                    stats = work.tile([128, 2, 6], f32, tag="st2")
                    for n in range(2):
                        ps = pss[n]
                        for kt in range(KT):
                            nc.tensor.matmul(
                                out=ps,
                                lhsT=attn_oT[:, kt, tt * 128:(tt + 1) * 128],
                                rhs=Wo_s[:, kt, n * 512:(n + 1) * 512],
                                start=(kt == 0),
                                stop=(kt == KT - 1 and triv_o))
                        if not triv_o:
                            nc.tensor.matmul(
                                out=ps, lhsT=ones_row[:, 0:128],
                                rhs=bo_s[:, n * 512:(n + 1) * 512],
                                start=False, stop=True)
                        nc.vector.bn_stats(out=stats[:, n, :], in_=ps)
                    mv = work.tile([128, 2], f32, tag="mv2")
                    nc.vector.bn_aggr(out=mv, in_=stats)
                    gc = gate_s[:, tt:tt + 1]
                    gv = work.tile([128, 1], f32, tag="gv")
                    nc.vector.tensor_mul(out=gv, in0=gc, in1=gc)
                    nc.vector.tensor_mul(out=gv, in0=gv, in1=mv[:, 1:2])
                    rstd = work.tile([128, 1], f32, tag="rs2")
                    nc.scalar.activation(out=rstd, in_=gv,
                                         func=AF.Sqrt, bias=eps_t)
                    nc.vector.reciprocal(out=rstd, in_=rstd)
                    sc = work.tile([128, 1], f32, tag="sc")
                    nc.vector.tensor_mul(out=sc, in0=rstd, in1=gc)
                    # (x - mu) * sc == x * sc + (-mu * sc): per-partition
                    # scale/bias lets the idle ACT engine do the big pass
                    mb = work.tile([128, 1], f32, tag="mb")
                    nc.vector.tensor_mul(out=mb, in0=mv[:, 0:1], in1=sc)
                    nc.vector.tensor_scalar_mul(out=mb, in0=mb, scalar1=-1.0)
                    xc = work.tile([128, D], f32, tag="xc2")
                    for n in range(2):
                        nc.scalar.activation(
                            out=xc[:, n * 512:(n + 1) * 512], in_=pss[n],
                            func=AF.Identity, bias=mb, scale=sc)
                    if triv_lno:
                        res = xc
                    else:
                        res = work.tile([128, D], f32, tag="res")
                        nc.vector.tensor_mul(out=res, in0=xc, in1=lnog_b)
                        nc.vector.tensor_add(out=res, in0=res, in1=lnob_b)
                    nc.sync.dma_start(
                        out=out_d[tt * 128:(tt + 1) * 128, :], in_=res)

    nc.compile()
    return nc


def _maybe_enable_trace():
    """Install the axon NTFF profile hook if tracing was requested."""
    if not os.environ.get("BASS_KERNEL_TRACE"):
        return False
    try:
        import sys
        import types
        import antenv
        if "antenv.axon_hooks" not in sys.modules:
            mod = types.ModuleType("antenv.axon_hooks")
            mod._hook = None
            mod.set_axon_ntff_profile_hook = lambda h: setattr(mod, "_hook", h)
            mod.get_axon_ntff_profile_hook = lambda: mod._hook
            sys.modules["antenv.axon_hooks"] = mod
            antenv.axon_hooks = mod
        from antenv.axon_hooks import get_axon_ntff_profile_hook
        if get_axon_ntff_profile_hook() is None:
            from trn_agent_boot.trn_boot import _ntff_profile_via_ctypes
            from antenv.axon_hooks import set_axon_ntff_profile_hook
            set_axon_ntff_profile_hook(
                _ntff_profile_via_ctypes("/opt/axon/libaxon_pjrt.so"))
        return True
    except Exception:
        return False


def kernel(**inputs):
    from concourse import bass_utils

    f = lambda k: np.ascontiguousarray(np.asarray(inputs[k], dtype=np.float32))
    # build-time specialization: skip bias/beta folding work when the actual
    # values make it a no-op (the general path handles arbitrary inputs)
    z = lambda k: not np.any(f(k))
    triv_qk = z("ln_q_b") and z("ln_kv_b") and z("bq") and z("bk")
    triv_v = z("ln_kv_b") and z("bv")
    triv_o = z("bo")
    triv_lno = z("ln_o_b") and bool(np.all(f("ln_o_g") == 1.0))
    key = ("nc", triv_qk, triv_v, triv_o, triv_lno)
    if key not in _CACHE:
        _CACHE[key] = _build(triv_qk, triv_v, triv_o, triv_lno)
    nc = _CACHE[key]
    query, key, value = f("query"), f("key"), f("value")
    shared = {
        "Wq": f("Wq"), "Wk": f("Wk"), "Wv": f("Wv"), "Wo": f("Wo"),
        "Wg": f("Wg").reshape(D, 1),
        "bq": f("bq").reshape(1, D), "bk": f("bk").reshape(1, D),
        "bv": f("bv").reshape(1, D), "bo": f("bo").reshape(1, D),
        "lnqg": f("ln_q_g").reshape(KT, 128),
        "lnqb": f("ln_q_b").reshape(KT, 128),
        "lnkg": f("ln_kv_g").reshape(KT, 128),
        "lnkb": f("ln_kv_b").reshape(KT, 128),
        "lnog": f("ln_o_g").reshape(1, D),
        "lnob": f("ln_o_b").reshape(1, D),
    }
    in_maps = []
    for c in range(N_CORES):
        b, hh = c // 2, c % 2
        in_maps.append({
            "xq": np.ascontiguousarray(query[b, hh * SQ:(hh + 1) * SQ, :]),
            "xk": np.ascontiguousarray(key[b]),
            "xv": np.ascontiguousarray(value[b]),
            **shared,
        })

    trace = _maybe_enable_trace()
    kw = {}
    if trace:
        kw = dict(trace=True, trace_cores=[0])
    res = bass_utils.run_bass_kernel_spmd(
        nc, in_maps, core_ids=list(range(N_CORES)), **kw)
    if trace:
        _CACHE["exec_time_ns"] = res.exec_time_ns
        _CACHE["trace_path"] = (res.instructions_and_trace[1]
                                if res.instructions_and_trace else None)

    out = np.empty((B, S, D), dtype=np.float32)
    for c in range(N_CORES):
        b, hh = c // 2, c % 2
        out[b, hh * SQ:(hh + 1) * SQ, :] = res.results[c]["out"]
    return out



# revision 25
# speedup vs baseline: 1.9648x; 1.0075x over previous
"""EnhancedMultiHeadAttention on 8 TRN2 NeuronCores.

Sharding: core c handles batch b=c//2 and query-row half h=c%2.
Each core computes the full attention for its 1024 query rows against its
batch's full 2048 keys/values (k/v work duplicated across the 2 cores that
share a batch — cheaper than an all-reduce). Outputs are disjoint slices of
the full [4, 2048, 1024] result, assembled on the host.

Kernel structure per core (bf16 matmuls, f32 softmax/LN):
  - LayerNorm in token-major layout; gain/beta folded into projection
    weights/biases (W' = diag(g) @ W, b' = beta @ W + b) so the normalized
    activations can be PE-transposed once and used directly.
  - q/k projections produce transposed outputs [D_out, tokens]; v is
    token-major with a ones column appended per head so the A@V matmul also
    yields the softmax denominator for free.
  - Scores are computed transposed [Sk, Sq]; exp (no max subtraction --
    scores are ~N(0,1) after scaling, bounded well inside f32 range) writes
    bf16 "E^T" tiles. A@V uses v as the stationary operand and E^T moving
    (N=512 matmuls), accumulating out^T [65, 1024] per head in PSUM; row 64
    is the softmax denominator, applied via a PSUM->SBUF copy +
    reciprocal_approx_fast (5x faster than the iterative reciprocal) +
    partition-broadcast + multiply, writing attn_out^T in the out-proj
    layout.
  - All work pools are shared across the k/v/q paths so the paths pipeline
    into each other instead of serializing on SBUF address reuse.
"""

import os
import numpy as np

D = 1024
H = 16
HD = 64
S = 2048
B = 4
SQ = 1024  # query rows per core
SK = 2048  # kv rows per core
KT = D // 128  # contraction tiles
MT = D // 128  # output chunks
N_CORES = 8
EPS = 1e-5

_CACHE = {}


def _build(triv_qk=False, triv_v=False, triv_o=False, triv_lno=False):
    """triv_* = the corresponding LN beta and projection bias are all zero
    (and for triv_lno, ln_o gain is all ones): skip the folded-bias work.
    The general path stays available for arbitrary inputs."""
    from contextlib import ExitStack

    import concourse.bacc as bacc
    import concourse.bass as bass
    import concourse.mybir as mybir
    import concourse.tile as tile
    from concourse.masks import make_identity

    f32 = mybir.dt.float32
    bf16 = mybir.dt.bfloat16
    AF = mybir.ActivationFunctionType
    OP = mybir.AluOpType

    nc = bacc.Bacc("TRN2", target_bir_lowering=False, debug=False,
                   num_devices=N_CORES)

    xq = nc.dram_tensor("xq", [SQ, D], f32, kind="ExternalInput").ap()
    xk = nc.dram_tensor("xk", [SK, D], f32, kind="ExternalInput").ap()
    xv = nc.dram_tensor("xv", [SK, D], f32, kind="ExternalInput").ap()
    Wq_d = nc.dram_tensor("Wq", [D, D], f32, kind="ExternalInput").ap()
    Wk_d = nc.dram_tensor("Wk", [D, D], f32, kind="ExternalInput").ap()
    Wv_d = nc.dram_tensor("Wv", [D, D], f32, kind="ExternalInput").ap()
    Wo_d = nc.dram_tensor("Wo", [D, D], f32, kind="ExternalInput").ap()
    Wg_d = nc.dram_tensor("Wg", [D, 1], f32, kind="ExternalInput").ap()
    bq_d = nc.dram_tensor("bq", [1, D], f32, kind="ExternalInput").ap()
    bk_d = nc.dram_tensor("bk", [1, D], f32, kind="ExternalInput").ap()
    bv_d = nc.dram_tensor("bv", [1, D], f32, kind="ExternalInput").ap()
    bo_d = nc.dram_tensor("bo", [1, D], f32, kind="ExternalInput").ap()
    lnqg_d = nc.dram_tensor("lnqg", [KT, 128], f32, kind="ExternalInput").ap()
    lnqb_d = nc.dram_tensor("lnqb", [KT, 128], f32, kind="ExternalInput").ap()
    lnkg_d = nc.dram_tensor("lnkg", [KT, 128], f32, kind="ExternalInput").ap()
    lnkb_d = nc.dram_tensor("lnkb", [KT, 128], f32, kind="ExternalInput").ap()
    lnog_d = nc.dram_tensor("lnog", [1, D], f32, kind="ExternalInput").ap()
    lnob_d = nc.dram_tensor("lnob", [1, D], f32, kind="ExternalInput").ap()
    out_d = nc.dram_tensor("out", [SQ, D], f32, kind="ExternalOutput").ap()

    def bcast_rows(ap2d, p):
        return bass.AP(tensor=ap2d.tensor, offset=ap2d.offset,
                       ap=[[0, p]] + list(ap2d.ap[1:]))

    with tile.TileContext(nc) as tc:
        with ExitStack() as ctx:
            const = ctx.enter_context(tc.tile_pool(name="const", bufs=1))
            main = ctx.enter_context(tc.tile_pool(name="main", bufs=1))
            wop = ctx.enter_context(tc.tile_pool(name="wo", bufs=1))

            identity = const.tile([128, 128], bf16)
            make_identity(nc, identity)
            ones_row = const.tile([1, 512], bf16)
            nc.vector.memset(ones_row, 1.0)
            eps_t = const.tile([128, 1], f32)
            nc.vector.memset(eps_t, EPS)

            lnqg = const.tile([128, KT], f32)
            nc.sync.dma_start(out=lnqg, in_=lnqg_d.rearrange("k p -> p k"))
            lnqb = const.tile([128, KT], f32)
            nc.sync.dma_start(out=lnqb, in_=lnqb_d.rearrange("k p -> p k"))
            lnkg = const.tile([128, KT], f32)
            nc.sync.dma_start(out=lnkg, in_=lnkg_d.rearrange("k p -> p k"))
            lnkb = const.tile([128, KT], f32)
            nc.sync.dma_start(out=lnkb, in_=lnkb_d.rearrange("k p -> p k"))
            lnqb_h = const.tile([128, KT], bf16)
            nc.vector.tensor_copy(out=lnqb_h, in_=lnqb)
            lnkb_h = const.tile([128, KT], bf16)
            nc.vector.tensor_copy(out=lnkb_h, in_=lnkb)

            # persistent per-core intermediates
            kT_s = main.tile([128, MT, SK], bf16)
            qT_s = main.tile([128, MT, SQ], bf16)
            v_aug = main.tile([128, SK // 128, H, HD + 1], bf16)
            gate_s = main.tile([128, SQ // 128], f32)
            nc.vector.memset(v_aug[:, :, :, HD:HD + 1], 1.0)
            # zero-padded qT staging (one per head parity): streaming K=128
            # keeps the PE activity monitor at full clock (K=64 matmuls get
            # permanently throttled to half rate).
            qtz = [main.tile([128, SQ], bf16, name=f"qtz{i}")
                   for i in range(2)]
            nc.vector.memset(qtz[0], 0.0)
            nc.vector.memset(qtz[1], 0.0)

            Wo_s = wop.tile([128, KT, D], bf16)
            bo_s = wop.tile([1, D], bf16)

            # ---------------- pre-attention ----------------
            with tc.tile_pool(name="wst", bufs=2) as wst, \
                    tc.tile_pool(name="wpl", bufs=2) as wpl, \
                    tc.tile_pool(name="xnt", bufs=3) as xnt, \
                    tc.tile_pool(name="lnw", bufs=4) as lnw, \
                    tc.tile_pool(name="bps", bufs=1, space="PSUM") as bps, \
                    tc.tile_pool(name="lps",
                                 bufs=(3 if triv_qk and triv_v else 2),
                                 space="PSUM") as lps, \
                    tc.tile_pool(name="pps",
                                 bufs=(4 if triv_qk and triv_v else 2),
                                 space="PSUM") as pps:

                def load_weight_folded(w_dram, b_dram, g, beta_h, name,
                                       Ws=None, bs=None, trivial=False):
                    """W' = diag(g) W (bf16), b' = beta @ W + b.
                    trivial=True: beta and b are all-zero, skip b' entirely."""
                    if Ws is None:
                        Ws = wpl.tile([128, KT, D], bf16, tag="W",
                                      name=f"{name}_W")
                        bs = wpl.tile([1, D], bf16, tag="b", name=f"{name}_b")
                    fold = g is not None and not trivial
                    if fold:
                        bp = bps.tile([1, 2, 512], f32, tag="bp",
                                      name=f"{name}_bp")
                    for kt in range(KT):
                        wc = wst.tile([128, D], f32, tag="wc",
                                      name=f"{name}_wc{kt}")
                        nc.sync.dma_start(
                            out=wc, in_=w_dram[kt * 128:(kt + 1) * 128, :])
                        if g is not None:
                            nc.scalar.activation(out=Ws[:, kt, :], in_=wc,
                                                 func=AF.Copy,
                                                 scale=g[:, kt:kt + 1])
                        else:
                            nc.scalar.activation(out=Ws[:, kt, :], in_=wc,
                                                 func=AF.Copy)
                        if fold:
                            raw = wst.tile([128, D], bf16, tag="raw",
                                           bufs=1, name=f"{name}_raw{kt}")
                            nc.scalar.copy(out=raw, in_=wc)
                            for n in range(2):
                                nc.tensor.matmul(
                                    out=bp[:, n, :],
                                    lhsT=beta_h[:, kt:kt + 1],
                                    rhs=raw[:, n * 512:(n + 1) * 512],
                                    start=(kt == 0), stop=(kt == KT - 1))
                    if trivial:
                        return Ws, bs
                    bb = wst.tile([1, D], f32, tag="bb", name=f"{name}_bb")
                    nc.sync.dma_start(out=bb, in_=b_dram)
                    if fold:
                        nc.vector.tensor_add(
                            out=bs, in0=bp.rearrange("p a b -> p (a b)"),
                            in1=bb)
                    else:
                        nc.vector.tensor_copy(out=bs, in_=bb)
                    return Ws, bs

                def bias_cols(bs, name):
                    """b' row [1, D] -> per-partition columns [128, MT] f32."""
                    bcp = bps.tile([128, MT], f32, tag="bg",
                                   name=f"{name}_bcp")
                    for m in range(MT):
                        nc.tensor.matmul(
                            out=bcp[:, m:m + 1],
                            lhsT=bs[:, m * 128:(m + 1) * 128],
                            rhs=ones_row[:, 0:1], start=True, stop=True)
                    bcol = wpl.tile([128, MT], f32, tag="bc",
                                    name=f"{name}_bcol")
                    nc.vector.tensor_copy(out=bcol, in_=bcp)
                    return bcol

                def ln_transpose(x_dram, n_tok, name):
                    """LN (no gain/beta) + PE transpose into per-512-token
                    chunk tiles [128, KT, 512] bf16."""
                    chunks = [xnt.tile([128, KT, 512], bf16, tag="xnt",
                                       name=f"{name}{i}")
                              for i in range(n_tok // 512)]
                    for t in range(n_tok // 128):
                        xt = lnw.tile([128, D], f32, tag="x")
                        nc.sync.dma_start(
                            out=xt, in_=x_dram[t * 128:(t + 1) * 128, :])
                        xt3 = xt.rearrange("p (s f) -> p s f", s=2)
                        stats = lnw.tile([128, 2, 6], f32, tag="st")
                        nc.vector.bn_stats(out=stats[:, 0, :], in_=xt3[:, 0, :])
                        nc.vector.bn_stats(out=stats[:, 1, :], in_=xt3[:, 1, :])
                        mv = lnw.tile([128, 2], f32, tag="mv")
                        nc.vector.bn_aggr(out=mv, in_=stats)
                        rstd = lnw.tile([128, 1], f32, tag="rs")
                        nc.scalar.activation(out=rstd, in_=mv[:, 1:2],
                                             func=AF.Sqrt, bias=eps_t)
                        nc.vector.reciprocal(out=rstd, in_=rstd)
                        xc = lnw.tile([128, D], bf16, tag="xc", bufs=2)
                        nc.vector.tensor_scalar(
                            out=xc, in0=xt, scalar1=mv[:, 0:1], scalar2=rstd,
                            op0=OP.subtract, op1=OP.mult)
                        pt = lps.tile([128, KT, 128], bf16)
                        for c in range(KT):
                            nc.tensor.transpose(
                                out=pt[:, c, :],
                                in_=xc[:, c * 128:(c + 1) * 128],
                                identity=identity)
                        cc, col = t // 4, (t % 4) * 128
                        nc.scalar.copy(
                            out=chunks[cc][:, :, col:col + 128], in_=pt)
                    return chunks

                def proj_T(xT_chunks, Ws, bcol, n_tok, dst):
                    """dst[:, m, n-chunk] = (W' xn^T + b'), bf16, transposed.
                    b' applied as a per-partition bias in the PSUM->SBUF copy."""
                    for n in range(n_tok // 512):
                        xT = xT_chunks[n]
                        for m in range(MT):
                            ps = pps.tile([128, 512], f32, tag="pj")
                            for kt in range(KT):
                                nc.tensor.matmul(
                                    out=ps,
                                    lhsT=Ws[:, kt, m * 128:(m + 1) * 128],
                                    rhs=xT[:, kt, :],
                                    start=(kt == 0), stop=(kt == KT - 1))
                            if bcol is None:
                                nc.scalar.copy(
                                    out=dst[:, m, n * 512:(n + 1) * 512],
                                    in_=ps)
                            else:
                                nc.scalar.activation(
                                    out=dst[:, m, n * 512:(n + 1) * 512],
                                    in_=ps, func=AF.Identity,
                                    bias=bcol[:, m:m + 1])

                # K path
                Wk_s, bk_s = load_weight_folded(Wk_d, bk_d, lnkg, lnkb_h,
                                                "wk", trivial=triv_qk)
                bcol_k = None if triv_qk else bias_cols(bk_s, "wk")
                knT = ln_transpose(xk, SK, "knT")
                proj_T(knT, Wk_s, bcol_k, SK, kT_s)

                # V path (bias applied as a broadcast row in the copy)
                Wv_s, bv_s = load_weight_folded(Wv_d, bv_d, lnkg, lnkb_h,
                                                "wv", trivial=triv_v)
                if not triv_v:
                    bvb = wpl.tile([128, D], bf16, tag="bvb", bufs=1)
                    nc.gpsimd.partition_broadcast(out_ap=bvb, in_ap=bv_s)
                vnT = ln_transpose(xv, SK, "vnT")
                for tt in range(SK // 128):
                    for n in range(2):
                        ps = pps.tile([128, 512], f32, tag="pj")
                        for kt in range(KT):
                            nc.tensor.matmul(
                                out=ps,
                                lhsT=vnT[tt // 4][
                                    :, kt, (tt % 4) * 128:(tt % 4 + 1) * 128],
                                rhs=Wv_s[:, kt, n * 512:(n + 1) * 512],
                                start=(kt == 0), stop=(kt == KT - 1))
                        if triv_v:
                            nc.vector.tensor_copy(
                                out=v_aug[:, tt, n * 8:(n + 1) * 8, 0:HD],
                                in_=ps.rearrange("p (h d) -> p h d", h=8))
                        else:
                            nc.vector.scalar_tensor_tensor(
                                out=v_aug[:, tt, n * 8:(n + 1) * 8, 0:HD],
                                in0=ps.rearrange("p (h d) -> p h d", h=8),
                                scalar=1.0, op0=OP.mult, op1=OP.add,
                                in1=bvb[:, n * 512:(n + 1) * 512].rearrange(
                                    "p (h d) -> p h d", h=8))

                # Q path (+ gate)
                Wq_s, bq_s = load_weight_folded(Wq_d, bq_d, lnqg, lnqb_h,
                                                "wq", trivial=triv_qk)
                Wg_s = const.tile([128, KT, 1], bf16)
                if not triv_qk:
                    bg_s = const.tile([1, 1], bf16)
                    bgp = bps.tile([1, 1], f32, tag="bg")
                for kt in range(KT):
                    gc = wst.tile([128, 1], f32, tag="gc", name=f"gc{kt}")
                    nc.sync.dma_start(
                        out=gc, in_=Wg_d[kt * 128:(kt + 1) * 128, :])
                    nc.scalar.activation(out=Wg_s[:, kt, :], in_=gc,
                                         func=AF.Copy,
                                         scale=lnqg[:, kt:kt + 1])
                    if not triv_qk:
                        gr = wst.tile([128, 1], bf16, tag="gr",
                                      name=f"gr{kt}")
                        nc.vector.tensor_copy(out=gr, in_=gc)
                        nc.tensor.matmul(out=bgp,
                                         lhsT=lnqb_h[:, kt:kt + 1],
                                         rhs=gr, start=(kt == 0),
                                         stop=(kt == KT - 1))
                if not triv_qk:
                    nc.vector.tensor_copy(out=bg_s, in_=bgp)

                qnT = ln_transpose(xq, SQ, "qnT")
                proj_T(qnT, Wq_s,
                       None if triv_qk else bias_cols(bq_s, "wq"), SQ, qT_s)
                for tt in range(SQ // 128):
                    ps = pps.tile([128, 1], f32, tag="g", bufs=1)
                    for kt in range(KT):
                        nc.tensor.matmul(
                            out=ps,
                            lhsT=qnT[tt // 4][
                                :, kt, (tt % 4) * 128:(tt % 4 + 1) * 128],
                            rhs=Wg_s[:, kt, :],
                            start=(kt == 0), stop=(kt == KT - 1 and triv_qk))
                    if not triv_qk:
                        nc.tensor.matmul(
                            out=ps, lhsT=ones_row[:, 0:128], rhs=bg_s,
                            start=False, stop=True)
                    nc.scalar.activation(
                        out=gate_s[:, tt:tt + 1], in_=ps, func=AF.Exp,
                        scale=-1.0)

                nc.vector.tensor_scalar_add(out=gate_s, in0=gate_s,
                                            scalar1=1.0)
                nc.vector.reciprocal_approx_fast(out=gate_s, in_=gate_s)

                # output projection weights (no LN folding)
                load_weight_folded(Wo_d, bo_d, None, None, "wo",
                                   Ws=Wo_s, bs=bo_s, trivial=triv_o)

            # ---------------- attention ----------------
            aop = ctx.enter_context(tc.tile_pool(name="aop", bufs=1))
            attn_oT = aop.tile([128, KT, SQ], bf16)
            with tc.tile_pool(name="psS", bufs=2, space="PSUM") as psS, \
                    tc.tile_pool(name="psO", bufs=2, space="PSUM") as psO, \
                    tc.tile_pool(name="et", bufs=3) as etp, \
                    tc.tile_pool(name="dv", bufs=2) as dvp:
                for h in range(H):
                    mch = h // 2
                    poh = (h % 2) * HD
                    ET = [etp.tile([128, 8, SQ], bf16, tag="et",
                                   name=f"et_h{h}_{i}") for i in range(2)]
                    pO = psO.tile([65, 2, 512], f32, tag="o",
                                  name=f"po_h{h}")
                    qz = qtz[h % 2]
                    nc.vector.tensor_copy(out=qz[poh:poh + HD, :],
                                          in_=qT_s[poh:poh + HD, mch, :])
                    for half in range(2):
                        for si in range(8):
                            sk = half * 8 + si
                            ps = psS.tile([128, SQ], f32, tag="s",
                                          name=f"ps_h{h}_{sk}")
                            for n in range(SQ // 512):
                                nc.tensor.matmul(
                                    out=ps[:, n * 512:(n + 1) * 512],
                                    lhsT=kT_s[:, mch,
                                              sk * 128:(sk + 1) * 128],
                                    rhs=qz[:, n * 512:(n + 1) * 512],
                                    start=True, stop=True)
                            nc.scalar.activation(
                                out=ET[half][:, si, :], in_=ps, func=AF.Exp,
                                scale=0.125)
                        for si in range(8):
                            sk = half * 8 + si
                            for n in range(2):
                                nc.tensor.matmul(
                                    out=pO[:, n, :],
                                    lhsT=v_aug[:, sk, h, :],
                                    rhs=ET[half][:, si,
                                                 n * 512:(n + 1) * 512],
                                    start=(sk == 0), stop=(sk == 15))
                    # softmax denominator: row 64 of pO
                    rs = dvp.tile([1, SQ], f32, tag="rs", name=f"rs_h{h}")
                    nc.vector.tensor_copy(
                        out=rs, in_=pO[64:65, :, :].rearrange(
                            "p a b -> p (a b)"))
                    nc.vector.reciprocal_approx_fast(out=rs, in_=rs)
                    rb = dvp.tile([HD, SQ], f32, tag="rb", name=f"rb_h{h}")
                    nc.gpsimd.partition_broadcast(out_ap=rb, in_ap=rs)
                    nc.vector.tensor_mul(
                        out=attn_oT[poh:poh + HD, mch, :],
                        in0=pO[0:64, :, :].rearrange("p a b -> p (a b)"),
                        in1=rb)

            # ---------------- out-proj + gate + final LN ----------------
            with tc.tile_pool(name="o_ps", bufs=8, space="PSUM") as pps, \
                    tc.tile_pool(name="o_w", bufs=4) as work:
                if not triv_lno:
                    lnog_b = work.tile([128, D], f32, tag="lng", bufs=1)
                    nc.sync.dma_start(out=lnog_b,
                                      in_=bcast_rows(lnog_d, 128))
                    lnob_b = work.tile([128, D], f32, tag="lnb", bufs=1)
                    nc.sync.dma_start(out=lnob_b,
                                      in_=bcast_rows(lnob_d, 128))
                for tt in range(SQ // 128):
                    # gate is folded into the final LN: LN(c*x) =
                    # (x - mean(x)) * c/sqrt(c^2 var(x) + eps) * g + b, c > 0.
                    # Stats are computed on the ungated PSUM directly.
                    pss = [pps.tile([128, 512], f32, tag="ops",
                                    name=f"ops{tt}_{n}") for n in range(2)]
                    stats = work.tile([128, 2, 6], f32, tag="st2")
                    for n in range(2):
                        ps = pss[n]
                        for kt in range(KT):
                            nc.tensor.matmul(
                                out=ps,
                                lhsT=attn_oT[:, kt, tt * 128:(tt + 1) * 128],
                                rhs=Wo_s[:, kt, n * 512:(n + 1) * 512],
                                start=(kt == 0),
                                stop=(kt == KT - 1 and triv_o))
                        if not triv_o:
                            nc.tensor.matmul(
                                out=ps, lhsT=ones_row[:, 0:128],
                                rhs=bo_s[:, n * 512:(n + 1) * 512],
                                start=False, stop=True)
                        nc.vector.bn_stats(out=stats[:, n, :], in_=ps)
                    mv = work.tile([128, 2], f32, tag="mv2")
                    nc.vector.bn_aggr(out=mv, in_=stats)
                    gc = gate_s[:, tt:tt + 1]
                    gv = work.tile([128, 1], f32, tag="gv")
                    nc.vector.tensor_mul(out=gv, in0=gc, in1=gc)
                    nc.vector.tensor_mul(out=gv, in0=gv, in1=mv[:, 1:2])
                    rstd = work.tile([128, 1], f32, tag="rs2")
                    nc.scalar.activation(out=rstd, in_=gv,
                                         func=AF.Sqrt, bias=eps_t)
                    nc.vector.reciprocal(out=rstd, in_=rstd)
                    sc = work.tile([128, 1], f32, tag="sc")
                    nc.vector.tensor_mul(out=sc, in0=rstd, in1=gc)
                    # (x - mu) * sc == x * sc + (-mu * sc): per-partition
                    # scale/bias lets the idle ACT engine do the big pass
                    mb = work.tile([128, 1], f32, tag="mb")
                    nc.vector.tensor_mul(out=mb, in0=mv[:, 0:1], in1=sc)
                    nc.vector.tensor_scalar_mul(out=mb, in0=mb, scalar1=-1.0)
                    xc = work.tile([128, D], f32, tag="xc2")
                    for n in range(2):
                        nc.scalar.activation(
                            out=xc[:, n * 512:(n + 1) * 512], in_=pss[n],
                            func=AF.Identity, bias=mb, scale=sc)
                    if triv_lno:
                        res = xc
                    else:
                        res = work.tile([128, D], f32, tag="res")
                        nc.vector.tensor_mul(out=res, in0=xc, in1=lnog_b)
                        nc.vector.tensor_add(out=res, in0=res, in1=lnob_b)
                    nc.sync.dma_start(
                        out=out_d[tt * 128:(tt + 1) * 128, :], in_=res)

    nc.compile()
    return nc


def _maybe_enable_trace():
    """Install the axon NTFF profile hook if tracing was requested."""
    if not os.environ.get("BASS_KERNEL_TRACE"):
        return False
    try:
        import sys
        import types
        import antenv
        if "antenv.axon_hooks" not in sys.modules:
            mod = types.ModuleType("antenv.axon_hooks")
            mod._hook = None
            mod.set_axon_ntff_profile_hook = lambda h: setattr(mod, "_hook", h)
            mod.get_axon_ntff_profile_hook = lambda: mod._hook
            sys.modules["antenv.axon_hooks"] = mod
            antenv.axon_hooks = mod
        from antenv.axon_hooks import get_axon_ntff_profile_hook
        if get_axon_ntff_profile_hook() is None:
            from trn_agent_boot.trn_boot import _ntff_profile_via_ctypes
            from antenv.axon_hooks import set_axon_ntff_profile_hook
            set_axon_ntff_profile_hook(
                _ntff_profile_via_ctypes("/opt/axon/libaxon_pjrt.so"))
        return True
    except Exception:
        return False


def kernel(**inputs):
    from concourse import bass_utils

    f = lambda k: np.ascontiguousarray(np.asarray(inputs[k], dtype=np.float32))
    # build-time specialization: skip bias/beta folding work when the actual
    # values make it a no-op (the general path handles arbitrary inputs)
    z = lambda k: not np.any(f(k))
    triv_qk = z("ln_q_b") and z("ln_kv_b") and z("bq") and z("bk")
    triv_v = z("ln_kv_b") and z("bv")
    triv_o = z("bo")
    triv_lno = z("ln_o_b") and bool(np.all(f("ln_o_g") == 1.0))
    key = ("nc", triv_qk, triv_v, triv_o, triv_lno)
    if key not in _CACHE:
        _CACHE[key] = _build(triv_qk, triv_v, triv_o, triv_lno)
    nc = _CACHE[key]
    query, key, value = f("query"), f("key"), f("value")
    shared = {
        "Wq": f("Wq"), "Wk": f("Wk"), "Wv": f("Wv"), "Wo": f("Wo"),
        "Wg": f("Wg").reshape(D, 1),
        "bq": f("bq").reshape(1, D), "bk": f("bk").reshape(1, D),
        "bv": f("bv").reshape(1, D), "bo": f("bo").reshape(1, D),
        "lnqg": f("ln_q_g").reshape(KT, 128),
        "lnqb": f("ln_q_b").reshape(KT, 128),
        "lnkg": f("ln_kv_g").reshape(KT, 128),
        "lnkb": f("ln_kv_b").reshape(KT, 128),
        "lnog": f("ln_o_g").reshape(1, D),
        "lnob": f("ln_o_b").reshape(1, D),
    }
    in_maps = []
    for c in range(N_CORES):
        b, hh = c // 2, c % 2
        in_maps.append({
            "xq": np.ascontiguousarray(query[b, hh * SQ:(hh + 1) * SQ, :]),
            "xk": np.ascontiguousarray(key[b]),
            "xv": np.ascontiguousarray(value[b]),
            **shared,
        })

    trace = _maybe_enable_trace()
    kw = {}
    if trace:
        kw = dict(trace=True, trace_cores=[0])
    res = bass_utils.run_bass_kernel_spmd(
        nc, in_maps, core_ids=list(range(N_CORES)), **kw)
    if trace:
        _CACHE["exec_time_ns"] = res.exec_time_ns
        _CACHE["trace_path"] = (res.instructions_and_trace[1]
                                if res.instructions_and_trace else None)

    out = np.empty((B, S, D), dtype=np.float32)
    for c in range(N_CORES):
        b, hh = c // 2, c % 2
        out[b, hh * SQ:(hh + 1) * SQ, :] = res.results[c]["out"]
    return out

